# revision 58
# baseline (speedup 1.0000x reference)
"""Trainium2 Bass kernel for a prenorm transformer Block (B=8, N=1024, D=768,
12 heads, MLP hidden 3072), data-parallel over batch across 8 NeuronCores.

v3. Engine/queue-level restructure of v2:

  - Dual DMA queues: bulk weight prefetch (wproj/x16/w1/w2, 9.4MB) triggers on
    the Activation HWDGE queue right after the qk phase, so the Sync queue and
    its rings stay clear for the latency-critical small DMAs (softmax
    denominator gathers/broadcasts) during attention. v2 issued w1/w2 on the
    sync queue *after* the attention stream, so their transfers only started
    near attention end and the fc1 phase raced weight arrival.
  - Scores matmuls contract 64 real K rows per head (even head on PE rows
    0:63, odd on rows 64:127 via inferred tile_position) -- no zero-padded K
    planes, no 5us memset, k-drain is one copy instead of two.
  - Softmax exp merged: one [128, 2x1024] Exp per (pair, token-tile) instead
    of two [128,1024] (saves the ~470ns fixed cost per ACT op; the exp is the
    scalar-engine bottleneck of the attention phase). q/k are drained
    unscaled; the fp8 dequant scale is folded into the Exp's scale operand.
  - Denominator rows gather SBUF->SBUF into adjacent partitions (no DRAM
    round trip before the Ln/Exp reciprocal).
  - proj contracts ctx chunks 0..4 for four output chunks first (4 concurrent
    PSUM groups) and the den-gated chunk 5 last, so the PE has ~4.3us of work
    queued while the last head pair's denominator chain completes.
  - LN finishes are emitted *after* the next matmul block so their a/b
    broadcast matmuls never stall the in-order PE stream (fin1 after fc1(nb0),
    fin2(nb0) two et-groups into fc2(nb1)); LN stats chain loses two scalar
    ops (mean scale folded into a -1/D ones-row on the b-broadcast matmul,
    E[x^2] scale moved to DVE).
  - Output is fp16 (converted to f32 host-side), written with one merged
    rearranged DMA per token half.
"""
import sys
import types

sys.path.insert(0, "/opt/trn_rl_repo")

try:
    import antenv.axon_hooks  # noqa: F401
except Exception:
    try:
        import antenv

        _hooks = types.ModuleType("antenv.axon_hooks")
        _hooks._hook = None

        def _set_hook(h):
            _hooks._hook = h

        def _get_hook():
            return _hooks._hook

        _hooks.set_axon_ntff_profile_hook = _set_hook
        _hooks.get_axon_ntff_profile_hook = _get_hook
        sys.modules["antenv.axon_hooks"] = _hooks
        antenv.axon_hooks = _hooks
    except Exception:
        pass

import ml_dtypes
import numpy as np

import concourse.bass as bass
import concourse.tile as tile
from concourse import mybir
from concourse.bass_utils import run_bass_kernel_spmd

F32R = mybir.dt.float32r
F32 = mybir.dt.float32
F16 = mybir.dt.float16
F8 = mybir.dt.float8e4
DR = mybir.MatmulPerfMode.DoubleRow
AF = mybir.ActivationFunctionType
OP = mybir.AluOpType
XS, WS = 16.0, 256.0                 # fp8 scales: x, weights

NCORES = 8
D, HEADS, HID, N = 768, 12, 3072, 1024
HD = D // HEADS                  # 64 head dim
DC = D // 128                    # 6 feature chunks
FC = HID // 128                  # 24 hidden chunks
NB = N // 512                    # 2 moving-dim blocks
MT = N // 128                    # 8 token tiles
NPR = HEADS // 2                 # 6 head pairs
EPS = 1e-6
SOFT_SCALE = 1.0 / (XS * WS) ** 2

# packed per-feature constants: column offsets in constsC
CBPJ, CBF1, CBF2, CG1, CB1, CG2, CB2 = 0, 6, 30, 36, 42, 48, 54

LAST_RESULT = None


# The walrus build in this container rejects instructions carrying more than
# a couple of sync waits; hoist excess waits onto standalone EventSemaphore
# carriers on the same engine (semantically identical).
_MM_OPS = ("Matmult", "Ldweights")


def _split_excess_waits(nc, default_limit=1, matmul_limit=0):
    counter = 0
    for f in nc.m.functions:
        for bb in f.blocks:
            new_insts = []
            for inst in bb.instructions:
                si = inst.sync_info
                waits = list(si.on_wait) if si and si.on_wait else []
                limit = matmul_limit if inst.opcode in _MM_OPS else default_limit
                if len(waits) > limit:
                    keep, move = waits[:limit], waits[limit:]
                    for w in move:
                        counter += 1
                        ev = mybir.InstEventSemaphore(
                            name=f"I-waitsplit-{counter}",
                            engine=inst.engine,
                            sync_info=mybir.SyncInfo(on_wait=[w], on_update=[]),
                        )
                        nc.register_instruction(ev, overwrite=True)
                        new_insts.append(ev)
                    inst.sync_info = mybir.SyncInfo(
                        on_wait=keep, on_update=list(si.on_update) if si else []
                    )
                new_insts.append(inst)
            bb.instructions = new_insts
    return counter


def _build(trivial_affine=False):
    nc = bass.Bass()

    xT16 = nc.dram_tensor("xT16", [D, N], F16, kind="ExternalInput")
    xT8 = nc.dram_tensor("xT8", [D, N], F8, kind="ExternalInput")
    wqkvT = nc.dram_tensor("wqkvT", [D, 3 * D], F8, kind="ExternalInput")
    wprojT = nc.dram_tensor("wprojT", [D, D], F16, kind="ExternalInput")
    wfc1T = nc.dram_tensor("wfc1T", [D, HID], F8, kind="ExternalInput")
    wfc2T = nc.dram_tensor("wfc2T", [HID, D], F8, kind="ExternalInput")
    constsC = nc.dram_tensor("constsC", [128, 60], F32, kind="ExternalInput")
    yT16 = nc.dram_tensor("yT16", [D, N], F16, kind="ExternalOutput")

    with tile.TileContext(nc) as tc:
        # ---- long-lived left-side pools (pushed first, released last) ----
        const = tc.alloc_tile_pool(name="const", bufs=1)
        stats = tc.alloc_tile_pool(name="stats", bufs=1)
        p_x1 = tc.alloc_tile_pool(name="p_x1", bufs=1)
        p_sq = tc.alloc_tile_pool(name="p_sq", bufs=1)
        p_u = tc.alloc_tile_pool(name="p_u", bufs=2)
        dscr = tc.alloc_tile_pool(name="dscr", bufs=1, space="DRAM")

        onesrow = const.tile([1, 128], F32R)
        nc.vector.tensor_copy(onesrow[:], nc.const_aps.tensor(1.0, (1, 128)))
        # LN sum lhsT columns with the 1/D mean scale folded in: s1 = -mean,
        # s2 = E[x^2], so the finish chain needs no separate scale ops and
        # both a/b broadcasts use the same onesrow lhsT.
        m16 = const.tile([128, 1], F16)
        nc.vector.memset(m16[:], -1.0 / D)
        p16 = const.tile([128, 1], F16)
        nc.vector.memset(p16[:], 1.0 / D)
        eps_t = const.tile([1, 1], F32)
        nc.vector.memset(eps_t[:], EPS)
        consts_sb = const.tile([128, 60], F32)
        x1_sb = p_x1.tile([128, DC, N], F16)
        x18_sb = p_x1.tile([128, DC, N], F8)   # fp8 copy of x1 for the fc1 DR
        warm = stats.tile([1, 8], F32, tag="warm", name="warm")
        nc.vector.memset(warm[:], 1.0)
        nc.scalar.activation(out=warm[:], in_=warm[:], func=AF.Exp)

        # ------------- Phase 1: QKV projections -------------
        # startup DMAs on the sync queue in exact consumption order: v-column
        # weights + x8 first halves interleaved (the v matmuls are the
        # kernel's first tensor work), then x8 second halves, q cols, k cols.
        p_x16 = tc.alloc_tile_pool(name="p_x16", bufs=1)
        x16_sb = p_x16.tile([128, DC, N], F16)
        p_x8 = tc.alloc_tile_pool(name="p_x8", bufs=1)
        x8_sb = p_x8.tile([128, DC, N], F8)
        p_wqkv = tc.alloc_tile_pool(name="p_wqkv", bufs=1)
        wqkv_sb = p_wqkv.tile([128, DC, 3 * D], F8)

        nc.sync.dma_start(out=consts_sb[:], in_=constsC[:, :])
        for cp in range(DC // 2):
            for c in (2 * cp, 2 * cp + 1):
                nc.sync.dma_start(out=wqkv_sb[:, c, 2 * D:3 * D],
                                  in_=wqkvT[c * 128:(c + 1) * 128, 2 * D:3 * D])
            for c in (2 * cp, 2 * cp + 1):
                nc.sync.dma_start(out=x8_sb[:, c, 0:512],
                                  in_=xT8[c * 128:(c + 1) * 128, 0:512])
        for c in range(DC):
            nc.sync.dma_start(out=x8_sb[:, c, 512:N],
                              in_=xT8[c * 128:(c + 1) * 128, 512:N])
        for c in range(DC):
            nc.sync.dma_start(out=wqkv_sb[:, c, 0:D],
                              in_=wqkvT[c * 128:(c + 1) * 128, 0:D])
        for c in range(DC):
            nc.sync.dma_start(out=wqkv_sb[:, c, D:2 * D],
                              in_=wqkvT[c * 128:(c + 1) * 128, D:2 * D])

        p_qk = tc.alloc_tile_pool(name="p_qk", bufs=1, side="right")
        p_v = tc.alloc_tile_pool(name="p_v", bufs=1, side="right")
        q_sb = p_qk.tile([128, DC, N], F16)
        k_sb = p_qk.tile([128, DC, N], F16)
        v_sb = p_v.tile([128, MT, HEADS, HD + 1], F16)
        nc.vector.memset(v_sb[:, :, :, HD:HD + 1], 1.0)

        # v in direct layout: [token (partitions), v-dim]; drains alternate
        # vector/scalar (both idle here)
        ps_v = tc.alloc_tile_pool(name="ps_v", bufs=2, space="PSUM")
        for mt in range(MT):
            msl = slice(mt * 128, mt * 128 + 128)
            ps = ps_v.tile([128, D], F32, tag="v", name="psv")
            for j in range(DC // 2):
                nc.tensor.matmul(ps[:, 0:512], x8_sb[:, 2 * j:2 * j + 2, msl],
                                 wqkv_sb[:, 2 * j:2 * j + 2, 2 * D:2 * D + 512],
                                 start=(j == 0), stop=(j == DC // 2 - 1),
                                 perf_mode=DR)
                nc.tensor.matmul(ps[:, 512:768], x8_sb[:, 2 * j:2 * j + 2, msl],
                                 wqkv_sb[:, 2 * j:2 * j + 2, 2 * D + 512:3 * D],
                                 start=(j == 0), stop=(j == DC // 2 - 1),
                                 perf_mode=DR)
            vout = v_sb[:, mt, :, 0:HD]
            vin = ps[:].rearrange("p (h d) -> p h d", h=HEADS)
            if mt % 2 == 0:
                nc.vector.tensor_scalar_mul(vout, in0=vin, scalar1=1.0 / (XS * WS))
            else:
                nc.scalar.activation(out=vout, in_=vin, func=AF.Copy,
                                     scale=1.0 / (XS * WS))
        ps_v.release()

        # scores psum allocated BEFORE the qk pool so the first score
        # matmuls don't wait for the whole qk-phase psum to drain
        ps_s = tc.alloc_tile_pool(name="ps_s", bufs=1, space="PSUM")
        # q,k transposed: [qkv-row tile (partitions), tokens]; drains are
        # plain copies (fp8 dequant scale folded into the softmax Exp)
        ps_qk = tc.alloc_tile_pool(name="ps_qk", bufs=4, space="PSUM")
        for jt in [x for p in range(DC) for x in (p, DC + p)]:
            pr = jt % DC
            col0 = jt * 128
            for nb in range(NB):
                sl = slice(nb * 512, nb * 512 + 512)
                ps = ps_qk.tile([128, 512], F32, tag="qk", name="psqk")
                for j in range(DC // 2):
                    nc.tensor.matmul(ps[:], wqkv_sb[:, 2 * j:2 * j + 2, col0:col0 + 128],
                                     x8_sb[:, 2 * j:2 * j + 2, sl],
                                     start=(j == 0), stop=(j == DC // 2 - 1),
                                     perf_mode=DR)
                if jt < DC:
                    nc.vector.tensor_copy(q_sb[:, pr, sl], ps[:])
                else:
                    nc.scalar.activation(out=k_sb[:, pr, sl], in_=ps[:],
                                         func=AF.Copy)
        ps_qk.release()
        p_wqkv.release()
        p_x8.release()

        # bulk weight prefetch on the Activation HWDGE queue (separate rings
        # from the sync queue): fires right after the k drains, transfers
        # overlap the whole attention phase.
        p_wproj = tc.alloc_tile_pool(name="p_wproj", bufs=1)
        wproj_sb = p_wproj.tile([128, DC, D], F16)
        p_w2 = tc.alloc_tile_pool(name="p_w2", bufs=1)
        w2_sb = p_w2.tile([128, FC, D], F8)
        p_w1 = tc.alloc_tile_pool(name="p_w1", bufs=1)
        w1_sb = p_w1.tile([128, DC, HID], F8)
        with tc.tile_wait_until(0.028):
            nc.sync.dma_start(out=wproj_sb[:],
                              in_=wprojT[:, :].rearrange("(c p) d -> p c d", p=128))
            nc.sync.dma_start(out=x16_sb[:],
                              in_=xT16[:, :].rearrange("(c p) n -> p c n", p=128))
            nc.sync.dma_start(out=w1_sb[:],
                              in_=wfc1T[:, :].rearrange("(c p) h -> p c h", p=128))
            nc.sync.dma_start(out=w2_sb[:],
                              in_=wfc2T[:, :].rearrange("(f p) d -> p f d", p=128))

        # ------------- Phase 2: attention -------------
        p_ctx = tc.alloc_tile_pool(name="p_ctx", bufs=1)
        ctx_sb = p_ctx.tile([128, DC, N], F16)
        p_ae = tc.alloc_tile_pool(name="p_ae", bufs=6, side="right")
        p_craw = tc.alloc_tile_pool(name="p_craw", bufs=1, side="right")
        p_recb = tc.alloc_tile_pool(name="p_recb", bufs=2, side="right")
        ps_c = tc.alloc_tile_pool(name="ps_c", bufs=1, space="PSUM")

        craws = {}

        GROUPS = {2: (0, 6), 4: (6, 10), 5: (10, 12)}
        # per-group denominator gather tiles (base partition 0 to satisfy
        # the engine partition-quadrant rule); den rows DMA SBUF->SBUF
        # straight into their group row
        dgs = {}

        def grp_of(h):
            return 2 if h < 6 else (4 if h < 10 else 5)

        def den_group(g):
            """Batch-reciprocal denominators for a group of heads and
            normalize their context: one Ln + one Exp (same ACT table set
            as the softmax exp) instead of per-head reciprocals. The last
            group (a single head pair, latency-critical attention->proj
            transition) skips the gather: Ln/Exp run in place on each craw
            denominator row and all its DMA hops ride the Activation HWDGE
            queue so the triggers fire straight after the Exp."""
            h0, h1 = GROUPS[g]
            nh = h1 - h0
            dma_eng = nc.scalar if g == 5 else nc.sync
            rec_d = dscr.tile([6, N], F16, tag=f"recd{g}", name="rec_d")
            if g == 5:
                for i in range(nh):
                    row = craws[h0 + i][HD:HD + 1, :]
                    nc.scalar.activation(out=row, in_=row, func=AF.Ln)
                    nc.scalar.activation(out=row, in_=row, func=AF.Exp,
                                         scale=-1.0)
                    dma_eng.dma_start(out=rec_d[i:i + 1, :], in_=row)
            else:
                lng = stats.tile([6, N], F16, tag="lng", name="lng")
                nc.scalar.activation(out=lng[0:nh, :], in_=dgs[g][0:nh, :],
                                     func=AF.Ln)
                rec = stats.tile([6, N], F16, tag="rec", name="rec")
                nc.scalar.activation(out=rec[0:nh, :], in_=lng[0:nh, :],
                                     func=AF.Exp, scale=-1.0)
                dma_eng.dma_start(out=rec_d[0:nh, :], in_=rec[0:nh, :])
            for i in range(nh):
                h = h0 + i
                prh, h01 = h // 2, h % 2
                half = h01 * 64
                recb = p_recb.tile([64, N], F16, tag="recb", name="recb")
                dma_eng.dma_start(
                    out=recb[:],
                    in_=rec_d[i:i + 1, :].to_broadcast([64, N]))
                nc.vector.tensor_mul(ctx_sb[half:half + 64, prh, :],
                                     craws[h][0:HD, :], recb[:])

        cps = {}

        def emit_av(pr, mt, ae, ao):
            """attn@v for one (pr, mt) step; emitted two steps late so the
            PE's in-order stream stalls on neither the exp it depends on nor
            (at mt==0) the previous pair's craw drain of the accumulator
            banks. On the last token tile also fold in the per-pair
            epilogue: denominator rows are copied out first so the Ln/Exp
            reciprocal chain starts before the full context drain."""
            if mt == 0:
                for h01 in range(2):
                    cps[h01] = ps_c.tile([HD + 1, N], F32,
                                         tag=f"c{h01}", name=f"cps{h01}")
            for h01, at_t in ((0, ae), (1, ao)):
                h = 2 * pr + h01
                for nb in range(NB):
                    sl = slice(nb * 512, nb * 512 + 512)
                    nc.tensor.matmul(cps[h01][:, sl], v_sb[:, mt, h, :],
                                     at_t[:, sl],
                                     start=(mt == 0), stop=(mt == MT - 1))
            if mt == MT - 1:
                g = grp_of(2 * pr)
                if g != 5 and g not in dgs:
                    dgs[g] = stats.tile([6, N], F16, tag="dg", name=f"dg{g}")
                crs = {}
                for h01 in range(2):
                    h = 2 * pr + h01
                    craw = crs[h01] = p_craw.tile([HD + 1, N], F16,
                                                  tag=f"cr{h % 6}", name="craw")
                    craws[h] = craw
                    nc.vector.tensor_copy(craw[HD:HD + 1, :], cps[h01][HD:HD + 1, :])
                    if g != 5:
                        nc.sync.dma_start(out=dgs[g][h - GROUPS[g][0]:
                                                     h - GROUPS[g][0] + 1, :],
                                          in_=craw[HD:HD + 1, :])
                for h01 in range(2):
                    nc.vector.tensor_copy(crs[h01][0:HD, :], cps[h01][0:HD, :])
                if pr in GROUPS:
                    den_group(pr)

        pendings = []
        for pr in range(NPR):
            for mt in range(MT):
                msl = slice(mt * 128, mt * 128 + 128)
                pse = ps_s.tile([128, N], F32, tag="pse", name="pse")
                pso = ps_s.tile([128, N], F32, tag="pso", name="pso")
                for nb in range(NB):
                    sl = slice(nb * 512, nb * 512 + 512)
                    nc.tensor.matmul(pse[:, sl], k_sb[0:64, pr, msl],
                                     q_sb[0:64, pr, sl], start=True, stop=True)
                    nc.tensor.matmul(pso[:, sl], k_sb[64:128, pr, msl],
                                     q_sb[64:128, pr, sl], start=True, stop=True)
                ae = p_ae.tile([128, N], F16, tag="attnT", name="ae")
                ao = p_ae.tile([128, N], F16, tag="attnT", name="ao")
                nc.scalar.activation(out=ae[:], in_=pse[:], func=AF.Exp,
                                     scale=SOFT_SCALE)
                nc.scalar.activation(out=ao[:], in_=pso[:], func=AF.Exp,
                                     scale=SOFT_SCALE)
                pendings.append((pr, mt, ae, ao))
                if len(pendings) > 2:
                    emit_av(*pendings.pop(0))
        for p in pendings:
            emit_av(*p)
        ps_c.release()
        ps_s.release()
        p_recb.release()
        p_craw.release()
        p_ae.release()
        p_v.release()
        p_qk.release()

        # ------------- Phase 3+4: proj + LN1 + MLP + LN2 -------------
        p_r1 = tc.alloc_tile_pool(name="p_r1", bufs=1)
        r1_sb = p_r1.tile([128, DC, N], F16)
        p_y2 = tc.alloc_tile_pool(name="p_y2", bufs=1)
        p_h = tc.alloc_tile_pool(name="p_h", bufs=1)
        p_y16 = tc.alloc_tile_pool(name="p_y16", bufs=3)
        y2_sb = p_y2.tile([128, DC, N], F16)
        h_sb = p_h.tile([128, FC, N], F8)
        # ps_mm first so its four banks alias the score-psum banks (released
        # at the last exp read, ~2us before the attn@v accumulators drain) --
        # the proj up-front matmuls start that much earlier
        ps_mm = tc.alloc_tile_pool(name="ps_mm", bufs=4, space="PSUM")
        ps_ln = tc.alloc_tile_pool(name="ps_ln", bufs=1, space="PSUM")
        ps_ab = tc.alloc_tile_pool(name="ps_ab", bufs=1, space="PSUM")

        def ln_accum(st, src_sb, et, sl, first, last, sq_eng=None):
            """Fold chunk et of the pre-norm tensor into the LN sums. The s1/s2
            PSUM rows are allocated lazily on first call so the tag-rotating
            WAR dep lands after the previous user's finish chain is emitted.
            The square runs on GPSIMD (otherwise idle) for some chunks."""
            w = sl.stop - sl.start
            if "s1" not in st:
                st["s1"] = ps_ln.tile([1, 512], F32, tag="s1", name="s1")
                st["s2"] = ps_ln.tile([1, 512], F32, tag="s2", name="s2")
            sqt = p_u.tile([128, 512], F16, tag="sqt", name="sqt")
            eng = sq_eng or (nc.gpsimd if et in (1, 4) else nc.vector)
            eng.tensor_mul(sqt[:, 0:w], src_sb[:, et, sl], src_sb[:, et, sl])
            nc.tensor.matmul(st["s1"][:, 0:w], m16[:], src_sb[:, et, sl],
                             start=first, stop=last)
            nc.tensor.matmul(st["s2"][:, 0:w], p16[:], sqt[:, 0:w],
                             start=first, stop=last)

        def ln_finish(st, src_sb, sl, goff, boff, out_sb, out_f16=None,
                      out_dram=None, aff_eng=None, cast_eng=None,
                      split_out=False, out_f8=None):
            """Stats chain + affine for token slice sl. rsqrt(var+eps) is
            Exp(-0.5*Ln(.)) so everything stays in the nl-exp ACT set.
            Scalar chain is 3 ops (Square, Ln, Exp); the -1/D mean scale is
            folded into the negrow lhsT of the b-broadcast matmul. aff_eng
            picks the engine for the big affine ops (gpsimd for the
            non-critical LN2 half so the vector engine stays clear for the
            tail); cast_eng=scalar moves the a/b PSUM->SBUF cast off the
            vector engine; split_out pipelines the affine with the output
            DMA in two 3-chunk halves."""
            w = sl.stop - sl.start
            s1, s2 = st["s1"][:, 0:w], st["s2"][:, 0:w]
            ae_ = aff_eng or nc.vector
            t4 = stats.tile([1, 512], F32, tag="t4", name="t4")[:, 0:w]
            t2 = stats.tile([1, 512], F32, tag="t2", name="t2")[:, 0:w]
            t31f = stats.tile([1, 2, 512], F32R, tag="t31", name="t31")
            t3, t1 = t31f[:, 0, 0:w], t31f[:, 1, 0:w]  # a=1/std, b=-mu/std
            nc.scalar.activation(out=t4, in_=s1, func=AF.Square)  # mean^2
            nc.vector.tensor_sub(t2, s2, t4)                      # var
            nc.scalar.activation(out=t4, in_=t2, func=AF.Ln, bias=eps_t[:])
            nc.scalar.activation(out=t3, in_=t4, func=AF.Exp,
                                 scale=-0.5)                     # rsqrt(var+eps)
            nc.vector.tensor_mul(t1, s1, t3.bitcast(F32))
            abp = ps_ab.tile([128, 2, 512], F32, tag="abp", name="abp")
            if w <= 256:
                # one broadcast matmul for both a and b rows: out packs
                # [128, 2, w] inside a single PSUM bank
                abpv = abp[:, 0, 0:2 * w].rearrange("p (t w) -> p t w", t=2)
                nc.tensor.matmul(abpv, onesrow[:], t31f[:, :, 0:w],
                                 start=True, stop=True)
            else:
                abpv = abp[:, :, 0:w]
                nc.tensor.matmul(abp[:, 0, 0:w], onesrow[:], t3,
                                 start=True, stop=True)
                nc.tensor.matmul(abp[:, 1, 0:w], onesrow[:], t1,
                                 start=True, stop=True)
            abps = stats.tile([128, 2, 512], F16, tag=f"abps{sl.start}",
                              name="abps")
            if cast_eng is nc.scalar:
                nc.scalar.activation(out=abps[:, :, 0:w], in_=abpv,
                                     func=AF.Copy)
            else:
                (cast_eng or nc.vector).tensor_copy(abps[:, :, 0:w], abpv)

            def dma_half(half):
                cs = slice(3 * half, 3 * half + 3)
                nc.sync.dma_start(
                    out=out_dram[384 * half:384 * half + 384, sl].rearrange(
                        "(c p) n -> p c n", p=128),
                    in_=out_f16[:, cs, 0:w])

            if trivial_affine:
                # gamma==1, beta==0: out = src*a + b for all 6 chunks in
                # fused DVE ops using zero-stride broadcast of the per-token
                # scale/shift rows
                uall = p_sq.tile([128, DC, 512], F16, tag=f"uall{sl.start}",
                                 name="uall")
                if split_out:
                    pieces = [slice(0, 3), slice(3, 6)]
                elif out_f8 is not None:
                    # 2-chunk pieces so the fc1 DR pairs can start on the
                    # first chunk pair before the whole affine is done
                    pieces = [slice(0, 2), slice(2, 4), slice(4, 6)]
                else:
                    pieces = [slice(0, DC)]
                for pi, cs in enumerate(pieces):
                    nch = cs.stop - cs.start
                    ab0 = abps[:, 0:1, 0:w].to_broadcast([128, nch, w])
                    ab1 = abps[:, 1:2, 0:w].to_broadcast([128, nch, w])
                    ae_.tensor_mul(uall[:, cs, 0:w], src_sb[:, cs, sl], ab0)
                    if out_f16 is not None:
                        ae_.tensor_add(out_f16[:, cs, 0:w], uall[:, cs, 0:w], ab1)
                        if out_dram is not None and split_out:
                            dma_half(pi)
                    else:
                        ae_.tensor_add(out_sb[:, cs, sl], uall[:, cs, 0:w], ab1)
                        if out_f8 is not None:
                            ae_.tensor_scalar_mul(out_f8[:, cs, sl],
                                                  in0=out_sb[:, cs, sl],
                                                  scalar1=XS)
            else:
                for c in range(DC):
                    u = p_u.tile([128, 512], F16, tag="u", name="u")
                    ae_.tensor_mul(u[:, 0:w], src_sb[:, c, sl], abps[:, 0, 0:w])
                    ae_.tensor_add(u[:, 0:w], u[:, 0:w], abps[:, 1, 0:w])
                    dst = out_f16[:, c, 0:w] if out_f16 is not None else out_sb[:, c, sl]
                    ae_.tensor_scalar(out=dst, in0=u[:, 0:w],
                                      scalar1=consts_sb[:, goff + c:goff + c + 1],
                                      scalar2=consts_sb[:, boff + c:boff + c + 1],
                                      op0=OP.mult, op1=OP.add)
                    if out_f8 is not None:
                        ae_.tensor_scalar_mul(out_f8[:, c, sl],
                                              in0=out_sb[:, c, sl], scalar1=XS)
                    if out_dram is not None and split_out and c in (2, 5):
                        dma_half(c // 3)
            if out_dram is not None and not split_out:
                nc.sync.dma_start(
                    out=out_dram[:, sl].rearrange("(c p) n -> p c n", p=128),
                    in_=out_f16[:, :, 0:w])

        def proj_block(nb, st, pe_extras, defer_accums, fin_hook=None):
            """proj matmuls for token half nb. ctx chunks 0..4 are contracted
            for four et groups up front (the den-gated chunk 5 last) so the
            PE has queued work while the last denominator chain completes.
            pe_extras: deferred PE ops (prev half's tail accums) emitted
            after the up-front groups. Returns this half's deferred accum
            lambdas (all 6 when defer_accums, else the last 4)."""
            sl = slice(nb * 512, nb * 512 + 512)

            def drain(et, ps):
                t = p_u.tile([128, 512], F16, tag="pt", name="pt")
                nc.scalar.activation(out=t[:], in_=ps[:], func=AF.Identity,
                                     bias=consts_sb[:, CBPJ + et:CBPJ + et + 1],
                                     scale=1.0)
                nc.vector.tensor_add(r1_sb[:, et, sl], t[:], x16_sb[:, et, sl])

            pss = {}
            for et in range(4):
                pss[et] = ps_mm.tile([128, 512], F32, tag="mm", name="pspj")
            for c in range(5):
                for et in range(4):
                    nc.tensor.matmul(pss[et][:],
                                     wproj_sb[:, c, et * 128:(et + 1) * 128],
                                     ctx_sb[:, c, sl],
                                     start=(c == 0), stop=False)
            for fn in pe_extras:
                fn()
            for et in range(4):
                nc.tensor.matmul(pss[et][:],
                                 wproj_sb[:, 5, et * 128:(et + 1) * 128],
                                 ctx_sb[:, 5, sl], start=False, stop=True)
                drain(et, pss[et])
            for et in (4, 5):
                ps = ps_mm.tile([128, 512], F32, tag="mm", name="pspj")
                for c in range(DC):
                    nc.tensor.matmul(ps[:], wproj_sb[:, c, et * 128:(et + 1) * 128],
                                     ctx_sb[:, c, sl],
                                     start=(c == 0), stop=(c == DC - 1))
                drain(et, ps)
                if not defer_accums:
                    ln_accum(st, r1_sb, et - 4, sl, et - 4 == 0, False)
            if fin_hook is not None:
                fin_hook()
            first = 2 if not defer_accums else 0
            return [lambda e=e: ln_accum(st, r1_sb, e, sl, e == 0, e == DC - 1)
                    for e in range(first, DC)]

        def fc1_block(nb, extras):
            sl = slice(nb * 512, nb * 512 + 512)
            for ftg in range(FC):
                ps = ps_mm.tile([128, 512], F32, tag="mm", name="psf1")
                for j in range(DC // 2):
                    nc.tensor.matmul(ps[:],
                                     w1_sb[:, 2 * j:2 * j + 2,
                                           ftg * 128:(ftg + 1) * 128],
                                     x18_sb[:, 2 * j:2 * j + 2, sl],
                                     start=(j == 0), stop=(j == DC // 2 - 1),
                                     perf_mode=DR)
                nc.scalar.activation(out=h_sb[:, ftg, sl], in_=ps[:], func=AF.Gelu,
                                     bias=consts_sb[:, CBF1 + ftg:CBF1 + ftg + 1],
                                     scale=1.0 / (XS * WS))
                if ftg in extras:
                    extras[ftg]()

        def fc2_group(sl, et):
            w = sl.stop - sl.start
            ps = ps_mm.tile([128, 512], F32, tag="mm", name="psf2")
            for j in range(FC // 2):
                nc.tensor.matmul(ps[:, 0:w],
                                 w2_sb[:, 2 * j:2 * j + 2, et * 128:(et + 1) * 128],
                                 h_sb[:, 2 * j:2 * j + 2, sl],
                                 start=(j == 0), stop=(j == FC // 2 - 1),
                                 perf_mode=DR)
            t = p_u.tile([128, 512], F16, tag="ft", name="ft")
            nc.scalar.activation(out=t[:, 0:w], in_=ps[:, 0:w], func=AF.Identity,
                                 bias=consts_sb[:, CBF2 + et:CBF2 + et + 1],
                                 scale=1.0 / WS)
            nc.vector.tensor_add(y2_sb[:, et, sl], t[:, 0:w], x1_sb[:, et, sl])

        stA, stB, stC = {}, {}, {}
        sl0, sl1 = slice(0, 512), slice(512, 1024)
        accA = proj_block(0, stA, [], defer_accums=False)
        # accums for et 2..5 of nb0 interleave into nb1's up-front groups and
        # fin(nb0) is emitted at nb1's block end; nb1 defers all its accums
        # into fc1(nb0) so the s1/s2 bank WAR lands after fin(nb0)'s reads,
        # and fin(nb1) lands mid-fc1(nb0) so x1(nb1) is ready well before
        # fc1(nb1) while its broadcast matmuls never stall the PE stream.
        accB = proj_block(1, stB, accA, defer_accums=True,
                          fin_hook=lambda: ln_finish(stA, r1_sb, sl0, CG1, CB1,
                                                     x1_sb, out_f8=x18_sb))

        def pair(fns):
            return lambda: [fn() for fn in fns]

        fc1_extras = {i: pair(accB[2 * i:2 * i + 2]) for i in range(3)}
        fc1_extras[15] = lambda: ln_finish(stB, r1_sb, sl1, CG1, CB1, x1_sb,
                                           out_f8=x18_sb)
        fc1_block(0, fc1_extras)
        fc1_block(1, {})

        # fc2 nb0: full 512 block, internal defer-by-2 accums for et 0..3
        for et in range(DC):
            fc2_group(sl0, et)
            if et >= 2:
                ln_accum(stC, y2_sb, et - 2, sl0, et - 2 == 0, False)
        # fc2 nb1 runs as two 256-token quarters so quarter 0's LN2 finish
        # overlaps quarter 1's matmuls and only the last quarter's stats
        # chain is exposed at the very end.
        stD0, stD1 = {}, {}
        slq0, slq1 = slice(512, 768), slice(768, 1024)
        y16a = p_y16.tile([128, DC, 512], F16, tag="y16", name="y16a")
        for et in range(DC):
            fc2_group(slq0, et)
            if et == 0:
                ln_accum(stC, y2_sb, 4, sl0, False, False)
            elif et == 1:
                ln_accum(stC, y2_sb, 5, sl0, False, True)
            elif et == 2:
                ln_finish(stC, y2_sb, sl0, CG2, CB2, None, out_f16=y16a,
                          out_dram=yT16, cast_eng=nc.scalar)
            else:
                ln_accum(stD0, y2_sb, et - 3, slq0, et - 3 == 0, False,
                         sq_eng=nc.vector)
        y16b = p_y16.tile([128, DC, 512], F16, tag="y16", name="y16b")
        for et in range(DC):
            fc2_group(slq1, et)
            if et < 3:
                ln_accum(stD0, y2_sb, et + 3, slq0, False, et == 2,
                         sq_eng=nc.vector)
            elif et == 3:
                ln_finish(stD0, y2_sb, slq0, CG2, CB2, None, out_f16=y16b,
                          out_dram=yT16, cast_eng=nc.scalar)
            else:
                ln_accum(stD1, y2_sb, et - 4, slq1, et - 4 == 0, False,
                         sq_eng=nc.vector)
        for e in (2, 3, 4, 5):
            ln_accum(stD1, y2_sb, e, slq1, False, e == DC - 1, sq_eng=nc.vector)
        y16c = p_y16.tile([128, DC, 512], F16, tag="y16", name="y16c")
        ln_finish(stD1, y2_sb, slq1, CG2, CB2, None, out_f16=y16c,
                  out_dram=yT16, cast_eng=nc.scalar, split_out=True)
        ps_ab.release()
        ps_ln.release()
        ps_mm.release()
        dscr.release()
        p_y16.release()
        p_h.release()
        p_y2.release()
        p_r1.release()
        p_ctx.release()
        p_w1.release()
        p_w2.release()
        p_wproj.release()
        p_x16.release()
        p_u.release()
        p_sq.release()
        p_x1.release()
        stats.release()
        const.release()
    return nc


_NC_CACHE = {}


def _get_nc(trivial_affine=False):
    nc = _NC_CACHE.get(trivial_affine)
    if nc is None:
        nc = _build(trivial_affine)
        _split_excess_waits(nc)
        _NC_CACHE[trivial_affine] = nc
    return nc


def kernel(x, w_qkv, w_proj, b_proj, w_fc1, b_fc1, w_fc2, b_fc2,
           gamma1, beta1, gamma2, beta2):
    global LAST_RESULT
    x = np.asarray(x, dtype=np.float32)
    w_qkv = np.asarray(w_qkv, dtype=np.float32)
    w_proj = np.asarray(w_proj, dtype=np.float32)
    b_proj = np.asarray(b_proj, dtype=np.float32)
    w_fc1 = np.asarray(w_fc1, dtype=np.float32)
    b_fc1 = np.asarray(b_fc1, dtype=np.float32)
    w_fc2 = np.asarray(w_fc2, dtype=np.float32)
    b_fc2 = np.asarray(b_fc2, dtype=np.float32)
    gamma1 = np.asarray(gamma1, dtype=np.float32)
    beta1 = np.asarray(beta1, dtype=np.float32)
    gamma2 = np.asarray(gamma2, dtype=np.float32)
    beta2 = np.asarray(beta2, dtype=np.float32)

    F8NP = ml_dtypes.float8_e4m3
    wqkv_scaled = w_qkv.copy()
    wqkv_scaled[:D] *= HD ** -0.5                  # fold attention scale into Q
    wqkvT = np.ascontiguousarray((wqkv_scaled.T * WS).astype(F8NP))
    wprojT = np.ascontiguousarray(w_proj.T.astype(np.float16))
    wfc1T = np.ascontiguousarray((w_fc1.T * WS).astype(F8NP))
    wfc2T = np.ascontiguousarray((w_fc2.T * WS).astype(F8NP))

    def cols(v, nchunks):
        return np.ascontiguousarray(v.reshape(nchunks, 128).T)

    constsC = np.ascontiguousarray(np.hstack([
        cols(b_proj, DC), cols(b_fc1, FC), cols(b_fc2, DC),
        cols(gamma1, DC), cols(beta1, DC), cols(gamma2, DC), cols(beta2, DC),
    ]).astype(np.float32))

    shared = {
        "wqkvT": wqkvT, "wprojT": wprojT, "wfc1T": wfc1T, "wfc2T": wfc2T,
        "constsC": constsC,
    }
    in_maps = []
    for b in range(NCORES):
        m = dict(shared)
        xt = np.ascontiguousarray(x[b].T)
        m["xT16"] = xt.astype(np.float16)
        m["xT8"] = (xt * XS).astype(F8NP)
        in_maps.append(m)

    trivial = (np.all(gamma1 == 1.0) and np.all(beta1 == 0.0)
               and np.all(gamma2 == 1.0) and np.all(beta2 == 0.0))
    nc = _get_nc(trivial_affine=bool(trivial))
    LAST_RESULT = run_bass_kernel_spmd(nc, in_maps, list(range(NCORES)))
    out = np.stack([np.ascontiguousarray(LAST_RESULT.results[b]["yT16"].T)
                    for b in range(NCORES)])
    return out.astype(np.float32)


# revision 62
# speedup vs baseline: 1.0266x; 1.0266x over previous
"""Trainium2 Bass kernel for a prenorm transformer Block (B=8, N=1024, D=768,
12 heads, MLP hidden 3072), data-parallel over batch across 8 NeuronCores.

v3. Engine/queue-level restructure of v2:

  - Dual DMA queues: bulk weight prefetch (wproj/x16/w1/w2, 9.4MB) triggers on
    the Activation HWDGE queue right after the qk phase, so the Sync queue and
    its rings stay clear for the latency-critical small DMAs (softmax
    denominator gathers/broadcasts) during attention. v2 issued w1/w2 on the
    sync queue *after* the attention stream, so their transfers only started
    near attention end and the fc1 phase raced weight arrival.
  - Scores matmuls contract 64 real K rows per head (even head on PE rows
    0:63, odd on rows 64:127 via inferred tile_position) -- no zero-padded K
    planes, no 5us memset, k-drain is one copy instead of two.
  - Softmax exp merged: one [128, 2x1024] Exp per (pair, token-tile) instead
    of two [128,1024] (saves the ~470ns fixed cost per ACT op; the exp is the
    scalar-engine bottleneck of the attention phase). q/k are drained
    unscaled; the fp8 dequant scale is folded into the Exp's scale operand.
  - Denominator rows gather SBUF->SBUF into adjacent partitions (no DRAM
    round trip before the Ln/Exp reciprocal).
  - proj contracts ctx chunks 0..4 for four output chunks first (4 concurrent
    PSUM groups) and the den-gated chunk 5 last, so the PE has ~4.3us of work
    queued while the last head pair's denominator chain completes.
  - LN finishes are emitted *after* the next matmul block so their a/b
    broadcast matmuls never stall the in-order PE stream (fin1 after fc1(nb0),
    fin2(nb0) two et-groups into fc2(nb1)); LN stats chain loses two scalar
    ops (mean scale folded into a -1/D ones-row on the b-broadcast matmul,
    E[x^2] scale moved to DVE).
  - Output is fp16 (converted to f32 host-side), written with one merged
    rearranged DMA per token half.
"""
import sys
import types

sys.path.insert(0, "/opt/trn_rl_repo")

try:
    import antenv.axon_hooks  # noqa: F401
except Exception:
    try:
        import antenv

        _hooks = types.ModuleType("antenv.axon_hooks")
        _hooks._hook = None

        def _set_hook(h):
            _hooks._hook = h

        def _get_hook():
            return _hooks._hook

        _hooks.set_axon_ntff_profile_hook = _set_hook
        _hooks.get_axon_ntff_profile_hook = _get_hook
        sys.modules["antenv.axon_hooks"] = _hooks
        antenv.axon_hooks = _hooks
    except Exception:
        pass

import ml_dtypes
import numpy as np

import concourse.bass as bass
import concourse.tile as tile
from concourse import mybir
from concourse.bass_utils import run_bass_kernel_spmd

F32R = mybir.dt.float32r
F32 = mybir.dt.float32
F16 = mybir.dt.float16
F8 = mybir.dt.float8e4
DR = mybir.MatmulPerfMode.DoubleRow
AF = mybir.ActivationFunctionType
OP = mybir.AluOpType
XS, WS = 16.0, 256.0                 # fp8 scales: x, weights

NCORES = 8
D, HEADS, HID, N = 768, 12, 3072, 1024
HD = D // HEADS                  # 64 head dim
DC = D // 128                    # 6 feature chunks
FC = HID // 128                  # 24 hidden chunks
NB = N // 512                    # 2 moving-dim blocks
MT = N // 128                    # 8 token tiles
NPR = HEADS // 2                 # 6 head pairs
EPS = 1e-6
SOFT_SCALE = 1.0 / (XS * WS) ** 2

# packed per-feature constants: column offsets in constsC
CBPJ, CBF1, CBF2, CG1, CB1, CG2, CB2 = 0, 6, 30, 36, 42, 48, 54

LAST_RESULT = None


# The walrus build in this container rejects instructions carrying more than
# a couple of sync waits; hoist excess waits onto standalone EventSemaphore
# carriers on the same engine (semantically identical).
_MM_OPS = ("Matmult", "Ldweights")


def _split_excess_waits(nc, default_limit=1, matmul_limit=0):
    counter = 0
    for f in nc.m.functions:
        for bb in f.blocks:
            new_insts = []
            for inst in bb.instructions:
                si = inst.sync_info
                waits = list(si.on_wait) if si and si.on_wait else []
                limit = matmul_limit if inst.opcode in _MM_OPS else default_limit
                if len(waits) > limit:
                    keep, move = waits[:limit], waits[limit:]
                    for w in move:
                        counter += 1
                        ev = mybir.InstEventSemaphore(
                            name=f"I-waitsplit-{counter}",
                            engine=inst.engine,
                            sync_info=mybir.SyncInfo(on_wait=[w], on_update=[]),
                        )
                        nc.register_instruction(ev, overwrite=True)
                        new_insts.append(ev)
                    inst.sync_info = mybir.SyncInfo(
                        on_wait=keep, on_update=list(si.on_update) if si else []
                    )
                new_insts.append(inst)
            bb.instructions = new_insts
    return counter


def _build(trivial_affine=False):
    nc = bass.Bass()

    xT16 = nc.dram_tensor("xT16", [D, N], F16, kind="ExternalInput")
    xT8 = nc.dram_tensor("xT8", [D, N], F8, kind="ExternalInput")
    wqkvT = nc.dram_tensor("wqkvT", [D, 3 * D], F8, kind="ExternalInput")
    wprojT = nc.dram_tensor("wprojT", [D, D], F16, kind="ExternalInput")
    wfc1T = nc.dram_tensor("wfc1T", [D, HID], F8, kind="ExternalInput")
    wfc2T = nc.dram_tensor("wfc2T", [HID, D], F8, kind="ExternalInput")
    constsC = nc.dram_tensor("constsC", [128, 60], F32, kind="ExternalInput")
    yT16 = nc.dram_tensor("yT16", [D, N], F16, kind="ExternalOutput")

    with tile.TileContext(nc) as tc:
        # ---- long-lived left-side pools (pushed first, released last) ----
        const = tc.alloc_tile_pool(name="const", bufs=1)
        stats = tc.alloc_tile_pool(name="stats", bufs=1)
        p_x1 = tc.alloc_tile_pool(name="p_x1", bufs=1)
        p_sq = tc.alloc_tile_pool(name="p_sq", bufs=1)
        p_u = tc.alloc_tile_pool(name="p_u", bufs=2)
        dscr = tc.alloc_tile_pool(name="dscr", bufs=1, space="DRAM")

        onesrow = const.tile([1, 128], F32R)
        nc.vector.tensor_copy(onesrow[:], nc.const_aps.tensor(1.0, (1, 128)))
        # LN sum lhsT columns with the 1/D mean scale folded in: s1 = -mean,
        # s2 = E[x^2], so the finish chain needs no separate scale ops and
        # both a/b broadcasts use the same onesrow lhsT.
        m16 = const.tile([128, 1], F16)
        nc.vector.memset(m16[:], -1.0 / D)
        p16 = const.tile([128, 1], F16)
        nc.vector.memset(p16[:], 1.0 / D)
        # ones row on partitions 64(+) for the PE-broadcast of the last
        # head pair's softmax reciprocal (contraction row = craw's den row)
        oneshi = const.tile([128, 64], F16)
        nc.vector.memset(oneshi[64:66, :], 1.0)
        eps_t = const.tile([1, 1], F32)
        nc.vector.memset(eps_t[:], EPS)
        consts_sb = const.tile([128, 60], F32)
        x1_sb = p_x1.tile([128, DC, N], F16)
        x18_sb = p_x1.tile([128, DC, N], F8)   # fp8 copy of x1 for the fc1 DR
        warm = stats.tile([1, 8], F32, tag="warm", name="warm")
        nc.vector.memset(warm[:], 1.0)
        nc.scalar.activation(out=warm[:], in_=warm[:], func=AF.Exp)

        # ------------- Phase 1: QKV projections -------------
        # startup DMAs on the sync queue in exact consumption order: v-column
        # weights + x8 first halves interleaved (the v matmuls are the
        # kernel's first tensor work), then x8 second halves, q cols, k cols.
        p_x16 = tc.alloc_tile_pool(name="p_x16", bufs=1)
        x16_sb = p_x16.tile([128, DC, N], F16)
        p_x8 = tc.alloc_tile_pool(name="p_x8", bufs=1)
        x8_sb = p_x8.tile([128, DC, N], F8)
        p_wqkv = tc.alloc_tile_pool(name="p_wqkv", bufs=1)
        wqkv_sb = p_wqkv.tile([128, DC, 3 * D], F8)

        nc.sync.dma_start(out=consts_sb[:], in_=constsC[:, :])
        for cp in range(DC // 2):
            for c in (2 * cp, 2 * cp + 1):
                nc.sync.dma_start(out=wqkv_sb[:, c, 2 * D:3 * D],
                                  in_=wqkvT[c * 128:(c + 1) * 128, 2 * D:3 * D])
            for c in (2 * cp, 2 * cp + 1):
                nc.sync.dma_start(out=x8_sb[:, c, 0:512],
                                  in_=xT8[c * 128:(c + 1) * 128, 0:512])
        for c in range(DC):
            nc.sync.dma_start(out=x8_sb[:, c, 512:N],
                              in_=xT8[c * 128:(c + 1) * 128, 512:N])
        for c in range(DC):
            nc.sync.dma_start(out=wqkv_sb[:, c, 0:D],
                              in_=wqkvT[c * 128:(c + 1) * 128, 0:D])
        for c in range(DC):
            nc.sync.dma_start(out=wqkv_sb[:, c, D:2 * D],
                              in_=wqkvT[c * 128:(c + 1) * 128, D:2 * D])

        p_qk = tc.alloc_tile_pool(name="p_qk", bufs=1, side="right")
        p_v = tc.alloc_tile_pool(name="p_v", bufs=1, side="right")
        q_sb = p_qk.tile([128, DC, N], F16)
        k_sb = p_qk.tile([128, DC, N], F16)
        v_sb = p_v.tile([128, MT, HEADS, HD + 1], F16)
        nc.vector.memset(v_sb[:, :, :, HD:HD + 1], 1.0)

        # v in direct layout: [token (partitions), v-dim]; drains alternate
        # vector/scalar (both idle here)
        ps_v = tc.alloc_tile_pool(name="ps_v", bufs=2, space="PSUM")
        for mt in range(MT):
            msl = slice(mt * 128, mt * 128 + 128)
            ps = ps_v.tile([128, D], F32, tag="v", name="psv")
            for j in range(DC // 2):
                nc.tensor.matmul(ps[:, 0:512], x8_sb[:, 2 * j:2 * j + 2, msl],
                                 wqkv_sb[:, 2 * j:2 * j + 2, 2 * D:2 * D + 512],
                                 start=(j == 0), stop=(j == DC // 2 - 1),
                                 perf_mode=DR)
                nc.tensor.matmul(ps[:, 512:768], x8_sb[:, 2 * j:2 * j + 2, msl],
                                 wqkv_sb[:, 2 * j:2 * j + 2, 2 * D + 512:3 * D],
                                 start=(j == 0), stop=(j == DC // 2 - 1),
                                 perf_mode=DR)
            vout = v_sb[:, mt, :, 0:HD]
            vin = ps[:].rearrange("p (h d) -> p h d", h=HEADS)
            if mt % 2 == 0:
                nc.vector.tensor_scalar_mul(vout, in0=vin, scalar1=1.0 / (XS * WS))
            else:
                nc.scalar.activation(out=vout, in_=vin, func=AF.Copy,
                                     scale=1.0 / (XS * WS))
        ps_v.release()

        # scores psum allocated BEFORE the qk pool so the first score
        # matmuls don't wait for the whole qk-phase psum to drain
        ps_s = tc.alloc_tile_pool(name="ps_s", bufs=1, space="PSUM")
        # q,k transposed: [qkv-row tile (partitions), tokens]; drains are
        # plain copies (fp8 dequant scale folded into the softmax Exp)
        ps_qk = tc.alloc_tile_pool(name="ps_qk", bufs=4, space="PSUM")
        for jt in [x for p in range(DC) for x in (p, DC + p)]:
            pr = jt % DC
            col0 = jt * 128
            for nb in range(NB):
                sl = slice(nb * 512, nb * 512 + 512)
                ps = ps_qk.tile([128, 512], F32, tag="qk", name="psqk")
                for j in range(DC // 2):
                    nc.tensor.matmul(ps[:], wqkv_sb[:, 2 * j:2 * j + 2, col0:col0 + 128],
                                     x8_sb[:, 2 * j:2 * j + 2, sl],
                                     start=(j == 0), stop=(j == DC // 2 - 1),
                                     perf_mode=DR)
                if jt < DC:
                    nc.vector.tensor_copy(q_sb[:, pr, sl], ps[:])
                else:
                    nc.scalar.activation(out=k_sb[:, pr, sl], in_=ps[:],
                                         func=AF.Copy)
        ps_qk.release()
        p_wqkv.release()
        p_x8.release()

        # bulk weight prefetch on the Activation HWDGE queue (separate rings
        # from the sync queue): fires right after the k drains, transfers
        # overlap the whole attention phase.
        p_wproj = tc.alloc_tile_pool(name="p_wproj", bufs=1)
        wproj_sb = p_wproj.tile([128, DC, D], F16)
        p_w2 = tc.alloc_tile_pool(name="p_w2", bufs=1)
        w2_sb = p_w2.tile([128, FC, D], F8)
        p_w1 = tc.alloc_tile_pool(name="p_w1", bufs=1)
        w1_sb = p_w1.tile([128, DC, HID], F8)
        with tc.tile_wait_until(0.028):
            nc.sync.dma_start(out=wproj_sb[:],
                              in_=wprojT[:, :].rearrange("(c p) d -> p c d", p=128))
            nc.sync.dma_start(out=x16_sb[:],
                              in_=xT16[:, :].rearrange("(c p) n -> p c n", p=128))
            nc.sync.dma_start(out=w1_sb[:],
                              in_=wfc1T[:, :].rearrange("(c p) h -> p c h", p=128))
            nc.sync.dma_start(out=w2_sb[:],
                              in_=wfc2T[:, :].rearrange("(f p) d -> p f d", p=128))

        # ------------- Phase 2: attention -------------
        p_ctx = tc.alloc_tile_pool(name="p_ctx", bufs=1)
        ctx_sb = p_ctx.tile([128, DC, N], F16)
        p_ae = tc.alloc_tile_pool(name="p_ae", bufs=6, side="right")
        p_craw = tc.alloc_tile_pool(name="p_craw", bufs=1, side="right")
        p_recb = tc.alloc_tile_pool(name="p_recb", bufs=2, side="right")
        ps_c = tc.alloc_tile_pool(name="ps_c", bufs=1, space="PSUM")

        craws = {}

        GROUPS = {2: (0, 6), 4: (6, 10), 5: (10, 12)}
        # per-group denominator gather tiles (base partition 0 to satisfy
        # the engine partition-quadrant rule); den rows DMA SBUF->SBUF
        # straight into their group row
        dgs = {}

        def grp_of(h):
            return 2 if h < 6 else (4 if h < 10 else 5)

        def den_group(g):
            """Batch-reciprocal denominators for a group of heads and
            normalize their context: one Ln + one Exp (same ACT table set
            as the softmax exp) instead of per-head reciprocals."""
            h0, h1 = GROUPS[g]
            nh = h1 - h0
            lng = stats.tile([6, N], F16, tag="lng", name="lng")
            nc.scalar.activation(out=lng[0:nh, :], in_=dgs[g][0:nh, :],
                                 func=AF.Ln)
            rec = stats.tile([6, N], F16, tag="rec", name="rec")
            nc.scalar.activation(out=rec[0:nh, :], in_=lng[0:nh, :],
                                 func=AF.Exp, scale=-1.0)
            rec_d = dscr.tile([6, N], F16, tag=f"recd{g}", name="rec_d")
            nc.sync.dma_start(out=rec_d[0:nh, :], in_=rec[0:nh, :])
            for i in range(nh):
                h = h0 + i
                prh, h01 = h // 2, h % 2
                half = h01 * 64
                recb = p_recb.tile([64, N], F16, tag="recb", name="recb")
                nc.sync.dma_start(
                    out=recb[:],
                    in_=rec_d[i:i + 1, :].to_broadcast([64, N]))
                nc.vector.tensor_mul(ctx_sb[half:half + 64, prh, :],
                                     craws[h][0:HD, :], recb[:])

        def den_group5_pe():
            """Last head pair's normalization without any DMA hops: Ln/Exp in
            place on each craw denominator row, reciprocal broadcast to 64
            partitions by a PE matmul into the (still idle) a/b-broadcast
            PSUM bank, context scaled by DVE reads straight from PSUM.
            Emitted as a proj hook so the broadcast matmuls queue behind the
            proj up-front groups instead of stalling the PE."""
            for h in (10, 11):
                row = craws[h][HD:HD + 1, :]
                nc.scalar.activation(out=row, in_=row, func=AF.Ln)
                nc.scalar.activation(out=row, in_=row, func=AF.Exp, scale=-1.0)
            abpt = ps_ab.tile([128, 2, 512], F32, tag="abp", name="recbP")
            for nb in range(NB):
                sl = slice(nb * 512, nb * 512 + 512)
                po = nb * 64
                for i, h in ((0, 10), (1, 11)):
                    nc.tensor.matmul(abpt[po:po + 64, i, :],
                                     oneshi[64:65, :],
                                     craws[h][HD:HD + 1, sl],
                                     start=True, stop=True)
                for i, h in ((0, 10), (1, 11)):
                    prh, h01 = h // 2, h % 2
                    half = h01 * 64
                    nc.vector.tensor_mul(ctx_sb[half:half + 64, prh, sl],
                                         craws[h][0:HD, sl],
                                         abpt[po:po + 64, i, :])

        cps = {}

        def emit_av(pr, mt, ae, ao):
            """attn@v for one (pr, mt) step; emitted two steps late so the
            PE's in-order stream stalls on neither the exp it depends on nor
            (at mt==0) the previous pair's craw drain of the accumulator
            banks. On the last token tile also fold in the per-pair
            epilogue: denominator rows are copied out first so the Ln/Exp
            reciprocal chain starts before the full context drain."""
            if mt == 0:
                for h01 in range(2):
                    cps[h01] = ps_c.tile([HD + 1, N], F32,
                                         tag=f"c{h01}", name=f"cps{h01}")
            for h01, at_t in ((0, ae), (1, ao)):
                h = 2 * pr + h01
                for nb in range(NB):
                    sl = slice(nb * 512, nb * 512 + 512)
                    nc.tensor.matmul(cps[h01][:, sl], v_sb[:, mt, h, :],
                                     at_t[:, sl],
                                     start=(mt == 0), stop=(mt == MT - 1))
            if mt == MT - 1:
                g = grp_of(2 * pr)
                if g != 5 and g not in dgs:
                    dgs[g] = stats.tile([6, N], F16, tag="dg", name=f"dg{g}")
                for h01 in range(2):
                    h = 2 * pr + h01
                    craw = p_craw.tile([HD + 1, N], F16,
                                       tag=f"cr{h % 6}", name="craw")
                    craws[h] = craw
                    nc.vector.tensor_copy(craw[:], cps[h01][:])
                    if g != 5:
                        nc.sync.dma_start(out=dgs[g][h - GROUPS[g][0]:
                                                     h - GROUPS[g][0] + 1, :],
                                          in_=craw[HD:HD + 1, :])
                if pr in (2, 4):
                    den_group(pr)

        pendings = []
        for pr in range(NPR):
            for mt in range(MT):
                msl = slice(mt * 128, mt * 128 + 128)
                pse = ps_s.tile([128, N], F32, tag="pse", name="pse")
                pso = ps_s.tile([128, N], F32, tag="pso", name="pso")
                for nb in range(NB):
                    sl = slice(nb * 512, nb * 512 + 512)
                    nc.tensor.matmul(pse[:, sl], k_sb[0:64, pr, msl],
                                     q_sb[0:64, pr, sl], start=True, stop=True)
                    nc.tensor.matmul(pso[:, sl], k_sb[64:128, pr, msl],
                                     q_sb[64:128, pr, sl], start=True, stop=True)
                ae = p_ae.tile([128, N], F16, tag="attnT", name="ae")
                ao = p_ae.tile([128, N], F16, tag="attnT", name="ao")
                nc.scalar.activation(out=ae[:], in_=pse[:], func=AF.Exp,
                                     scale=SOFT_SCALE)
                nc.scalar.activation(out=ao[:], in_=pso[:], func=AF.Exp,
                                     scale=SOFT_SCALE)
                pendings.append((pr, mt, ae, ao))
                if len(pendings) > 2:
                    emit_av(*pendings.pop(0))
        for p in pendings:
            emit_av(*p)
        ps_c.release()
        ps_s.release()
        p_recb.release()
        p_craw.release()
        p_ae.release()
        p_v.release()
        p_qk.release()

        # ------------- Phase 3+4: proj + LN1 + MLP + LN2 -------------
        p_r1 = tc.alloc_tile_pool(name="p_r1", bufs=1)
        r1_sb = p_r1.tile([128, DC, N], F16)
        p_y2 = tc.alloc_tile_pool(name="p_y2", bufs=1)
        p_h = tc.alloc_tile_pool(name="p_h", bufs=1)
        p_y16 = tc.alloc_tile_pool(name="p_y16", bufs=3)
        y2_sb = p_y2.tile([128, DC, N], F16)
        h_sb = p_h.tile([128, FC, N], F8)
        # ps_mm first so its four banks alias the score-psum banks (released
        # at the last exp read, ~2us before the attn@v accumulators drain) --
        # the proj up-front matmuls start that much earlier
        ps_mm = tc.alloc_tile_pool(name="ps_mm", bufs=4, space="PSUM")
        ps_ln = tc.alloc_tile_pool(name="ps_ln", bufs=1, space="PSUM")
        ps_ab = tc.alloc_tile_pool(name="ps_ab", bufs=1, space="PSUM")

        def ln_accum(st, src_sb, et, sl, first, last, sq_eng=None):
            """Fold chunk et of the pre-norm tensor into the LN sums. The s1/s2
            PSUM rows are allocated lazily on first call so the tag-rotating
            WAR dep lands after the previous user's finish chain is emitted.
            The square runs on GPSIMD (otherwise idle) for some chunks."""
            w = sl.stop - sl.start
            if "s1" not in st:
                st["s1"] = ps_ln.tile([1, 512], F32, tag="s1", name="s1")
                st["s2"] = ps_ln.tile([1, 512], F32, tag="s2", name="s2")
            sqt = p_u.tile([128, 512], F16, tag="sqt", name="sqt")
            eng = sq_eng or (nc.gpsimd if et in (1, 4) else nc.vector)
            eng.tensor_mul(sqt[:, 0:w], src_sb[:, et, sl], src_sb[:, et, sl])
            nc.tensor.matmul(st["s1"][:, 0:w], m16[:], src_sb[:, et, sl],
                             start=first, stop=last)
            nc.tensor.matmul(st["s2"][:, 0:w], p16[:], sqt[:, 0:w],
                             start=first, stop=last)

        def ln_finish(st, src_sb, sl, goff, boff, out_sb, out_f16=None,
                      out_dram=None, aff_eng=None, cast_eng=None,
                      split_out=False, out_f8=None):
            """Stats chain + affine for token slice sl. rsqrt(var+eps) is
            Exp(-0.5*Ln(.)) so everything stays in the nl-exp ACT set.
            Scalar chain is 3 ops (Square, Ln, Exp); the -1/D mean scale is
            folded into the negrow lhsT of the b-broadcast matmul. aff_eng
            picks the engine for the big affine ops (gpsimd for the
            non-critical LN2 half so the vector engine stays clear for the
            tail); cast_eng=scalar moves the a/b PSUM->SBUF cast off the
            vector engine; split_out pipelines the affine with the output
            DMA in two 3-chunk halves."""
            w = sl.stop - sl.start
            s1, s2 = st["s1"][:, 0:w], st["s2"][:, 0:w]
            ae_ = aff_eng or nc.vector
            t4 = stats.tile([1, 512], F32, tag="t4", name="t4")[:, 0:w]
            t2 = stats.tile([1, 512], F32, tag="t2", name="t2")[:, 0:w]
            t31f = stats.tile([1, 2, 512], F32R, tag="t31", name="t31")
            t3, t1 = t31f[:, 0, 0:w], t31f[:, 1, 0:w]  # a=1/std, b=-mu/std
            nc.scalar.activation(out=t4, in_=s1, func=AF.Square)  # mean^2
            nc.vector.tensor_sub(t2, s2, t4)                      # var
            nc.scalar.activation(out=t4, in_=t2, func=AF.Ln, bias=eps_t[:])
            nc.scalar.activation(out=t3, in_=t4, func=AF.Exp,
                                 scale=-0.5)                     # rsqrt(var+eps)
            nc.vector.tensor_mul(t1, s1, t3.bitcast(F32))
            abp = ps_ab.tile([128, 2, 512], F32, tag="abp", name="abp")
            if w <= 256:
                # one broadcast matmul for both a and b rows: out packs
                # [128, 2, w] inside a single PSUM bank
                abpv = abp[:, 0, 0:2 * w].rearrange("p (t w) -> p t w", t=2)
                nc.tensor.matmul(abpv, onesrow[:], t31f[:, :, 0:w],
                                 start=True, stop=True)
            else:
                abpv = abp[:, :, 0:w]
                nc.tensor.matmul(abp[:, 0, 0:w], onesrow[:], t3,
                                 start=True, stop=True)
                nc.tensor.matmul(abp[:, 1, 0:w], onesrow[:], t1,
                                 start=True, stop=True)
            abps = stats.tile([128, 2, 512], F16, tag=f"abps{sl.start}",
                              name="abps")
            if cast_eng is nc.scalar:
                nc.scalar.activation(out=abps[:, :, 0:w], in_=abpv,
                                     func=AF.Copy)
            else:
                (cast_eng or nc.vector).tensor_copy(abps[:, :, 0:w], abpv)

            def dma_half(half):
                cs = slice(3 * half, 3 * half + 3)
                nc.sync.dma_start(
                    out=out_dram[384 * half:384 * half + 384, sl].rearrange(
                        "(c p) n -> p c n", p=128),
                    in_=out_f16[:, cs, 0:w])

            if trivial_affine:
                # gamma==1, beta==0: out = src*a + b for all 6 chunks in
                # fused DVE ops using zero-stride broadcast of the per-token
                # scale/shift rows
                uall = p_sq.tile([128, DC, 512], F16, tag=f"uall{sl.start}",
                                 name="uall")
                if split_out:
                    pieces = [slice(0, 3), slice(3, 6)]
                elif out_f8 is not None:
                    # 2-chunk pieces so the fc1 DR pairs can start on the
                    # first chunk pair before the whole affine is done
                    pieces = [slice(0, 2), slice(2, 4), slice(4, 6)]
                else:
                    pieces = [slice(0, DC)]
                for pi, cs in enumerate(pieces):
                    nch = cs.stop - cs.start
                    ab0 = abps[:, 0:1, 0:w].to_broadcast([128, nch, w])
                    ab1 = abps[:, 1:2, 0:w].to_broadcast([128, nch, w])
                    ae_.tensor_mul(uall[:, cs, 0:w], src_sb[:, cs, sl], ab0)
                    if out_f16 is not None:
                        ae_.tensor_add(out_f16[:, cs, 0:w], uall[:, cs, 0:w], ab1)
                        if out_dram is not None and split_out:
                            dma_half(pi)
                    else:
                        ae_.tensor_add(out_sb[:, cs, sl], uall[:, cs, 0:w], ab1)
                        if out_f8 is not None:
                            ae_.tensor_scalar_mul(out_f8[:, cs, sl],
                                                  in0=out_sb[:, cs, sl],
                                                  scalar1=XS)
            else:
                for c in range(DC):
                    u = p_u.tile([128, 512], F16, tag="u", name="u")
                    ae_.tensor_mul(u[:, 0:w], src_sb[:, c, sl], abps[:, 0, 0:w])
                    ae_.tensor_add(u[:, 0:w], u[:, 0:w], abps[:, 1, 0:w])
                    dst = out_f16[:, c, 0:w] if out_f16 is not None else out_sb[:, c, sl]
                    ae_.tensor_scalar(out=dst, in0=u[:, 0:w],
                                      scalar1=consts_sb[:, goff + c:goff + c + 1],
                                      scalar2=consts_sb[:, boff + c:boff + c + 1],
                                      op0=OP.mult, op1=OP.add)
                    if out_f8 is not None:
                        ae_.tensor_scalar_mul(out_f8[:, c, sl],
                                              in0=out_sb[:, c, sl], scalar1=XS)
                    if out_dram is not None and split_out and c in (2, 5):
                        dma_half(c // 3)
            if out_dram is not None and not split_out:
                nc.sync.dma_start(
                    out=out_dram[:, sl].rearrange("(c p) n -> p c n", p=128),
                    in_=out_f16[:, :, 0:w])

        def proj_block(nb, st, pe_extras, defer_accums, fin_hook=None):
            """proj matmuls for token half nb. ctx chunks 0..4 are contracted
            for four et groups up front (the den-gated chunk 5 last) so the
            PE has queued work while the last denominator chain completes.
            pe_extras: deferred PE ops (prev half's tail accums) emitted
            after the up-front groups. Returns this half's deferred accum
            lambdas (all 6 when defer_accums, else the last 4)."""
            sl = slice(nb * 512, nb * 512 + 512)

            def drain(et, ps):
                t = p_u.tile([128, 512], F16, tag="pt", name="pt")
                nc.scalar.activation(out=t[:], in_=ps[:], func=AF.Identity,
                                     bias=consts_sb[:, CBPJ + et:CBPJ + et + 1],
                                     scale=1.0)
                nc.vector.tensor_add(r1_sb[:, et, sl], t[:], x16_sb[:, et, sl])

            pss = {}
            for et in range(4):
                pss[et] = ps_mm.tile([128, 512], F32, tag="mm", name="pspj")
            for c in range(5):
                for et in range(4):
                    nc.tensor.matmul(pss[et][:],
                                     wproj_sb[:, c, et * 128:(et + 1) * 128],
                                     ctx_sb[:, c, sl],
                                     start=(c == 0), stop=False)
            for fn in pe_extras:
                fn()
            for et in range(4):
                nc.tensor.matmul(pss[et][:],
                                 wproj_sb[:, 5, et * 128:(et + 1) * 128],
                                 ctx_sb[:, 5, sl], start=False, stop=True)
                drain(et, pss[et])
            for et in (4, 5):
                ps = ps_mm.tile([128, 512], F32, tag="mm", name="pspj")
                for c in range(DC):
                    nc.tensor.matmul(ps[:], wproj_sb[:, c, et * 128:(et + 1) * 128],
                                     ctx_sb[:, c, sl],
                                     start=(c == 0), stop=(c == DC - 1))
                drain(et, ps)
                if not defer_accums:
                    ln_accum(st, r1_sb, et - 4, sl, et - 4 == 0, False)
            if fin_hook is not None:
                fin_hook()
            first = 2 if not defer_accums else 0
            return [lambda e=e: ln_accum(st, r1_sb, e, sl, e == 0, e == DC - 1)
                    for e in range(first, DC)]

        def fc1_block(nb, extras):
            sl = slice(nb * 512, nb * 512 + 512)
            for ftg in range(FC):
                ps = ps_mm.tile([128, 512], F32, tag="mm", name="psf1")
                for j in range(DC // 2):
                    nc.tensor.matmul(ps[:],
                                     w1_sb[:, 2 * j:2 * j + 2,
                                           ftg * 128:(ftg + 1) * 128],
                                     x18_sb[:, 2 * j:2 * j + 2, sl],
                                     start=(j == 0), stop=(j == DC // 2 - 1),
                                     perf_mode=DR)
                nc.scalar.activation(out=h_sb[:, ftg, sl], in_=ps[:], func=AF.Gelu,
                                     bias=consts_sb[:, CBF1 + ftg:CBF1 + ftg + 1],
                                     scale=1.0 / (XS * WS))
                if ftg in extras:
                    extras[ftg]()

        def fc2_group(sl, et):
            w = sl.stop - sl.start
            ps = ps_mm.tile([128, 512], F32, tag="mm", name="psf2")
            for j in range(FC // 2):
                nc.tensor.matmul(ps[:, 0:w],
                                 w2_sb[:, 2 * j:2 * j + 2, et * 128:(et + 1) * 128],
                                 h_sb[:, 2 * j:2 * j + 2, sl],
                                 start=(j == 0), stop=(j == FC // 2 - 1),
                                 perf_mode=DR)
            t = p_u.tile([128, 512], F16, tag="ft", name="ft")
            nc.scalar.activation(out=t[:, 0:w], in_=ps[:, 0:w], func=AF.Identity,
                                 bias=consts_sb[:, CBF2 + et:CBF2 + et + 1],
                                 scale=1.0 / WS)
            nc.vector.tensor_add(y2_sb[:, et, sl], t[:, 0:w], x1_sb[:, et, sl])

        stA, stB, stC = {}, {}, {}
        sl0, sl1 = slice(0, 512), slice(512, 1024)
        accA = proj_block(0, stA, [den_group5_pe], defer_accums=False)
        # accums for et 2..5 of nb0 interleave into nb1's up-front groups and
        # fin(nb0) is emitted at nb1's block end; nb1 defers all its accums
        # into fc1(nb0) so the s1/s2 bank WAR lands after fin(nb0)'s reads,
        # and fin(nb1) lands mid-fc1(nb0) so x1(nb1) is ready well before
        # fc1(nb1) while its broadcast matmuls never stall the PE stream.
        accB = proj_block(1, stB, accA, defer_accums=True,
                          fin_hook=lambda: ln_finish(stA, r1_sb, sl0, CG1, CB1,
                                                     x1_sb, out_f8=x18_sb))

        def pair(fns):
            return lambda: [fn() for fn in fns]

        fc1_extras = {i: pair(accB[2 * i:2 * i + 2]) for i in range(3)}
        fc1_extras[15] = lambda: ln_finish(stB, r1_sb, sl1, CG1, CB1, x1_sb,
                                           out_f8=x18_sb)
        fc1_block(0, fc1_extras)
        fc1_block(1, {})

        # fc2 nb0: full 512 block, internal defer-by-2 accums for et 0..3
        for et in range(DC):
            fc2_group(sl0, et)
            if et >= 2:
                ln_accum(stC, y2_sb, et - 2, sl0, et - 2 == 0, False)
        # fc2 nb1 runs as two 256-token quarters so quarter 0's LN2 finish
        # overlaps quarter 1's matmuls and only the last quarter's stats
        # chain is exposed at the very end.
        stD0, stD1 = {}, {}
        slq0, slq1 = slice(512, 768), slice(768, 1024)
        y16a = p_y16.tile([128, DC, 512], F16, tag="y16", name="y16a")
        for et in range(DC):
            fc2_group(slq0, et)
            if et == 0:
                ln_accum(stC, y2_sb, 4, sl0, False, False)
            elif et == 1:
                ln_accum(stC, y2_sb, 5, sl0, False, True)
            elif et == 2:
                ln_finish(stC, y2_sb, sl0, CG2, CB2, None, out_f16=y16a,
                          out_dram=yT16, cast_eng=nc.scalar)
            else:
                ln_accum(stD0, y2_sb, et - 3, slq0, et - 3 == 0, False,
                         sq_eng=nc.vector)
        y16b = p_y16.tile([128, DC, 512], F16, tag="y16", name="y16b")
        for et in range(DC):
            fc2_group(slq1, et)
            if et < 3:
                ln_accum(stD0, y2_sb, et + 3, slq0, False, et == 2,
                         sq_eng=nc.vector)
            elif et == 3:
                ln_finish(stD0, y2_sb, slq0, CG2, CB2, None, out_f16=y16b,
                          out_dram=yT16, cast_eng=nc.scalar)
            else:
                ln_accum(stD1, y2_sb, et - 4, slq1, et - 4 == 0, False,
                         sq_eng=nc.vector)
        for e in (2, 3, 4, 5):
            ln_accum(stD1, y2_sb, e, slq1, False, e == DC - 1, sq_eng=nc.vector)
        y16c = p_y16.tile([128, DC, 512], F16, tag="y16", name="y16c")
        ln_finish(stD1, y2_sb, slq1, CG2, CB2, None, out_f16=y16c,
                  out_dram=yT16, cast_eng=nc.scalar, split_out=True)
        ps_ab.release()
        ps_ln.release()
        ps_mm.release()
        dscr.release()
        p_y16.release()
        p_h.release()
        p_y2.release()
        p_r1.release()
        p_ctx.release()
        p_w1.release()
        p_w2.release()
        p_wproj.release()
        p_x16.release()
        p_u.release()
        p_sq.release()
        p_x1.release()
        stats.release()
        const.release()
    return nc


_NC_CACHE = {}


def _get_nc(trivial_affine=False):
    nc = _NC_CACHE.get(trivial_affine)
    if nc is None:
        nc = _build(trivial_affine)
        _split_excess_waits(nc)
        _NC_CACHE[trivial_affine] = nc
    return nc


def kernel(x, w_qkv, w_proj, b_proj, w_fc1, b_fc1, w_fc2, b_fc2,
           gamma1, beta1, gamma2, beta2):
    global LAST_RESULT
    x = np.asarray(x, dtype=np.float32)
    w_qkv = np.asarray(w_qkv, dtype=np.float32)
    w_proj = np.asarray(w_proj, dtype=np.float32)
    b_proj = np.asarray(b_proj, dtype=np.float32)
    w_fc1 = np.asarray(w_fc1, dtype=np.float32)
    b_fc1 = np.asarray(b_fc1, dtype=np.float32)
    w_fc2 = np.asarray(w_fc2, dtype=np.float32)
    b_fc2 = np.asarray(b_fc2, dtype=np.float32)
    gamma1 = np.asarray(gamma1, dtype=np.float32)
    beta1 = np.asarray(beta1, dtype=np.float32)
    gamma2 = np.asarray(gamma2, dtype=np.float32)
    beta2 = np.asarray(beta2, dtype=np.float32)

    F8NP = ml_dtypes.float8_e4m3
    wqkv_scaled = w_qkv.copy()
    wqkv_scaled[:D] *= HD ** -0.5                  # fold attention scale into Q
    wqkvT = np.ascontiguousarray((wqkv_scaled.T * WS).astype(F8NP))
    wprojT = np.ascontiguousarray(w_proj.T.astype(np.float16))
    wfc1T = np.ascontiguousarray((w_fc1.T * WS).astype(F8NP))
    wfc2T = np.ascontiguousarray((w_fc2.T * WS).astype(F8NP))

    def cols(v, nchunks):
        return np.ascontiguousarray(v.reshape(nchunks, 128).T)

    constsC = np.ascontiguousarray(np.hstack([
        cols(b_proj, DC), cols(b_fc1, FC), cols(b_fc2, DC),
        cols(gamma1, DC), cols(beta1, DC), cols(gamma2, DC), cols(beta2, DC),
    ]).astype(np.float32))

    shared = {
        "wqkvT": wqkvT, "wprojT": wprojT, "wfc1T": wfc1T, "wfc2T": wfc2T,
        "constsC": constsC,
    }
    in_maps = []
    for b in range(NCORES):
        m = dict(shared)
        xt = np.ascontiguousarray(x[b].T)
        m["xT16"] = xt.astype(np.float16)
        m["xT8"] = (xt * XS).astype(F8NP)
        in_maps.append(m)

    trivial = (np.all(gamma1 == 1.0) and np.all(beta1 == 0.0)
               and np.all(gamma2 == 1.0) and np.all(beta2 == 0.0))
    nc = _get_nc(trivial_affine=bool(trivial))
    LAST_RESULT = run_bass_kernel_spmd(nc, in_maps, list(range(NCORES)))
    out = np.stack([np.ascontiguousarray(LAST_RESULT.results[b]["yT16"].T)
                    for b in range(NCORES)])
    return out.astype(np.float32)


# revision 63
# speedup vs baseline: 1.0492x; 1.0220x over previous
"""Trainium2 Bass kernel for a prenorm transformer Block (B=8, N=1024, D=768,
12 heads, MLP hidden 3072), data-parallel over batch across 8 NeuronCores.

v3. Engine/queue-level restructure of v2:

  - Dual DMA queues: bulk weight prefetch (wproj/x16/w1/w2, 9.4MB) triggers on
    the Activation HWDGE queue right after the qk phase, so the Sync queue and
    its rings stay clear for the latency-critical small DMAs (softmax
    denominator gathers/broadcasts) during attention. v2 issued w1/w2 on the
    sync queue *after* the attention stream, so their transfers only started
    near attention end and the fc1 phase raced weight arrival.
  - Scores matmuls contract 64 real K rows per head (even head on PE rows
    0:63, odd on rows 64:127 via inferred tile_position) -- no zero-padded K
    planes, no 5us memset, k-drain is one copy instead of two.
  - Softmax exp merged: one [128, 2x1024] Exp per (pair, token-tile) instead
    of two [128,1024] (saves the ~470ns fixed cost per ACT op; the exp is the
    scalar-engine bottleneck of the attention phase). q/k are drained
    unscaled; the fp8 dequant scale is folded into the Exp's scale operand.
  - Denominator rows gather SBUF->SBUF into adjacent partitions (no DRAM
    round trip before the Ln/Exp reciprocal).
  - proj contracts ctx chunks 0..4 for four output chunks first (4 concurrent
    PSUM groups) and the den-gated chunk 5 last, so the PE has ~4.3us of work
    queued while the last head pair's denominator chain completes.
  - LN finishes are emitted *after* the next matmul block so their a/b
    broadcast matmuls never stall the in-order PE stream (fin1 after fc1(nb0),
    fin2(nb0) two et-groups into fc2(nb1)); LN stats chain loses two scalar
    ops (mean scale folded into a -1/D ones-row on the b-broadcast matmul,
    E[x^2] scale moved to DVE).
  - Output is fp16 (converted to f32 host-side), written with one merged
    rearranged DMA per token half.
"""
import sys
import types

sys.path.insert(0, "/opt/trn_rl_repo")

try:
    import antenv.axon_hooks  # noqa: F401
except Exception:
    try:
        import antenv

        _hooks = types.ModuleType("antenv.axon_hooks")
        _hooks._hook = None

        def _set_hook(h):
            _hooks._hook = h

        def _get_hook():
            return _hooks._hook

        _hooks.set_axon_ntff_profile_hook = _set_hook
        _hooks.get_axon_ntff_profile_hook = _get_hook
        sys.modules["antenv.axon_hooks"] = _hooks
        antenv.axon_hooks = _hooks
    except Exception:
        pass

import ml_dtypes
import numpy as np

import concourse.bass as bass
import concourse.tile as tile
from concourse import mybir
from concourse.bass_utils import run_bass_kernel_spmd

F32R = mybir.dt.float32r
F32 = mybir.dt.float32
F16 = mybir.dt.float16
F8 = mybir.dt.float8e4
DR = mybir.MatmulPerfMode.DoubleRow
AF = mybir.ActivationFunctionType
OP = mybir.AluOpType
XS, WS = 16.0, 256.0                 # fp8 scales: x, weights

NCORES = 8
D, HEADS, HID, N = 768, 12, 3072, 1024
HD = D // HEADS                  # 64 head dim
DC = D // 128                    # 6 feature chunks
FC = HID // 128                  # 24 hidden chunks
NB = N // 512                    # 2 moving-dim blocks
MT = N // 128                    # 8 token tiles
NPR = HEADS // 2                 # 6 head pairs
EPS = 1e-6
SOFT_SCALE = 1.0 / (XS * WS) ** 2

# packed per-feature constants: column offsets in constsC
CBPJ, CBF1, CBF2, CG1, CB1, CG2, CB2 = 0, 6, 30, 36, 42, 48, 54

LAST_RESULT = None


# The walrus build in this container rejects instructions carrying more than
# a couple of sync waits; hoist excess waits onto standalone EventSemaphore
# carriers on the same engine (semantically identical).
_MM_OPS = ("Matmult", "Ldweights")


def _split_excess_waits(nc, default_limit=1, matmul_limit=0):
    counter = 0
    for f in nc.m.functions:
        for bb in f.blocks:
            new_insts = []
            for inst in bb.instructions:
                si = inst.sync_info
                waits = list(si.on_wait) if si and si.on_wait else []
                limit = matmul_limit if inst.opcode in _MM_OPS else default_limit
                if len(waits) > limit:
                    keep, move = waits[:limit], waits[limit:]
                    for w in move:
                        counter += 1
                        ev = mybir.InstEventSemaphore(
                            name=f"I-waitsplit-{counter}",
                            engine=inst.engine,
                            sync_info=mybir.SyncInfo(on_wait=[w], on_update=[]),
                        )
                        nc.register_instruction(ev, overwrite=True)
                        new_insts.append(ev)
                    inst.sync_info = mybir.SyncInfo(
                        on_wait=keep, on_update=list(si.on_update) if si else []
                    )
                new_insts.append(inst)
            bb.instructions = new_insts
    return counter


def _build(trivial_affine=False):
    nc = bass.Bass()

    xT16 = nc.dram_tensor("xT16", [D, N], F16, kind="ExternalInput")
    xT8 = nc.dram_tensor("xT8", [D, N], F8, kind="ExternalInput")
    wqkvT = nc.dram_tensor("wqkvT", [D, 3 * D], F8, kind="ExternalInput")
    wprojT = nc.dram_tensor("wprojT", [D, D], F16, kind="ExternalInput")
    wfc1T = nc.dram_tensor("wfc1T", [D, HID], F8, kind="ExternalInput")
    wfc2T = nc.dram_tensor("wfc2T", [HID, D], F8, kind="ExternalInput")
    constsC = nc.dram_tensor("constsC", [128, 60], F32, kind="ExternalInput")
    yT16 = nc.dram_tensor("yT16", [D, N], F16, kind="ExternalOutput")

    with tile.TileContext(nc) as tc:
        # ---- long-lived left-side pools (pushed first, released last) ----
        const = tc.alloc_tile_pool(name="const", bufs=1)
        stats = tc.alloc_tile_pool(name="stats", bufs=1)
        p_x1 = tc.alloc_tile_pool(name="p_x1", bufs=1)
        p_sq = tc.alloc_tile_pool(name="p_sq", bufs=1)
        p_u = tc.alloc_tile_pool(name="p_u", bufs=2)
        dscr = tc.alloc_tile_pool(name="dscr", bufs=1, space="DRAM")

        onesrow = const.tile([1, 128], F32R)
        nc.vector.tensor_copy(onesrow[:], nc.const_aps.tensor(1.0, (1, 128)))
        # LN sum lhsT columns with the 1/D mean scale folded in: s1 = -mean,
        # s2 = E[x^2], so the finish chain needs no separate scale ops and
        # both a/b broadcasts use the same onesrow lhsT.
        m16 = const.tile([128, 1], F16)
        nc.vector.memset(m16[:], -1.0 / D)
        p16 = const.tile([128, 1], F16)
        nc.vector.memset(p16[:], 1.0 / D)
        # ones row on partitions 64(+) for the PE-broadcast of the last
        # head pair's softmax reciprocal (contraction row = craw's den row)
        oneshi = const.tile([128, 64], F16)
        nc.vector.memset(oneshi[64:66, :], 1.0)
        eps_t = const.tile([1, 1], F32)
        nc.vector.memset(eps_t[:], EPS)
        consts_sb = const.tile([128, 60], F32)
        x1_sb = p_x1.tile([128, DC, N], F16)
        x18_sb = p_x1.tile([128, DC, N], F8)   # fp8 copy of x1 for the fc1 DR
        warm = stats.tile([1, 8], F32, tag="warm", name="warm")
        nc.vector.memset(warm[:], 1.0)
        nc.scalar.activation(out=warm[:], in_=warm[:], func=AF.Exp)

        # ------------- Phase 1: QKV projections -------------
        # startup DMAs on the sync queue in exact consumption order: v-column
        # weights + x8 first halves interleaved (the v matmuls are the
        # kernel's first tensor work), then x8 second halves, q cols, k cols.
        p_x16 = tc.alloc_tile_pool(name="p_x16", bufs=1)
        x16_sb = p_x16.tile([128, DC, N], F16)
        p_x8 = tc.alloc_tile_pool(name="p_x8", bufs=1)
        x8_sb = p_x8.tile([128, DC, N], F8)
        p_wqkv = tc.alloc_tile_pool(name="p_wqkv", bufs=1)
        wqkv_sb = p_wqkv.tile([128, DC, 3 * D], F8)

        nc.sync.dma_start(out=consts_sb[:], in_=constsC[:, :])
        for cp in range(DC // 2):
            for c in (2 * cp, 2 * cp + 1):
                nc.sync.dma_start(out=wqkv_sb[:, c, 2 * D:3 * D],
                                  in_=wqkvT[c * 128:(c + 1) * 128, 2 * D:3 * D])
            for c in (2 * cp, 2 * cp + 1):
                nc.sync.dma_start(out=x8_sb[:, c, 0:512],
                                  in_=xT8[c * 128:(c + 1) * 128, 0:512])
        for c in range(DC):
            nc.sync.dma_start(out=x8_sb[:, c, 512:N],
                              in_=xT8[c * 128:(c + 1) * 128, 512:N])
        for c in range(DC):
            nc.sync.dma_start(out=wqkv_sb[:, c, 0:D],
                              in_=wqkvT[c * 128:(c + 1) * 128, 0:D])
        for c in range(DC):
            nc.sync.dma_start(out=wqkv_sb[:, c, D:2 * D],
                              in_=wqkvT[c * 128:(c + 1) * 128, D:2 * D])

        p_qk = tc.alloc_tile_pool(name="p_qk", bufs=1, side="right")
        p_v = tc.alloc_tile_pool(name="p_v", bufs=1, side="right")
        q_sb = p_qk.tile([128, DC, N], F16)
        k_sb = p_qk.tile([128, DC, N], F16)
        v_sb = p_v.tile([128, MT, HEADS, HD + 1], F16)
        nc.vector.memset(v_sb[:, :, :, HD:HD + 1], 1.0)

        # v in direct layout: [token (partitions), v-dim]; drains alternate
        # vector/scalar (both idle here)
        ps_v = tc.alloc_tile_pool(name="ps_v", bufs=2, space="PSUM")
        for mt in range(MT):
            msl = slice(mt * 128, mt * 128 + 128)
            ps = ps_v.tile([128, D], F32, tag="v", name="psv")
            for j in range(DC // 2):
                nc.tensor.matmul(ps[:, 0:512], x8_sb[:, 2 * j:2 * j + 2, msl],
                                 wqkv_sb[:, 2 * j:2 * j + 2, 2 * D:2 * D + 512],
                                 start=(j == 0), stop=(j == DC // 2 - 1),
                                 perf_mode=DR)
                nc.tensor.matmul(ps[:, 512:768], x8_sb[:, 2 * j:2 * j + 2, msl],
                                 wqkv_sb[:, 2 * j:2 * j + 2, 2 * D + 512:3 * D],
                                 start=(j == 0), stop=(j == DC // 2 - 1),
                                 perf_mode=DR)
            vout = v_sb[:, mt, :, 0:HD]
            vin = ps[:].rearrange("p (h d) -> p h d", h=HEADS)
            if mt % 2 == 0:
                nc.vector.tensor_scalar_mul(vout, in0=vin, scalar1=1.0 / (XS * WS))
            else:
                nc.scalar.activation(out=vout, in_=vin, func=AF.Copy,
                                     scale=1.0 / (XS * WS))
        ps_v.release()

        # scores psum allocated BEFORE the qk pool so the first score
        # matmuls don't wait for the whole qk-phase psum to drain
        ps_s = tc.alloc_tile_pool(name="ps_s", bufs=1, space="PSUM")
        # q,k transposed: [qkv-row tile (partitions), tokens]; drains are
        # plain copies (fp8 dequant scale folded into the softmax Exp)
        ps_qk = tc.alloc_tile_pool(name="ps_qk", bufs=4, space="PSUM")
        for jt in [x for p in range(DC) for x in (p, DC + p)]:
            pr = jt % DC
            col0 = jt * 128
            for nb in range(NB):
                sl = slice(nb * 512, nb * 512 + 512)
                ps = ps_qk.tile([128, 512], F32, tag="qk", name="psqk")
                for j in range(DC // 2):
                    nc.tensor.matmul(ps[:], wqkv_sb[:, 2 * j:2 * j + 2, col0:col0 + 128],
                                     x8_sb[:, 2 * j:2 * j + 2, sl],
                                     start=(j == 0), stop=(j == DC // 2 - 1),
                                     perf_mode=DR)
                if jt < DC:
                    nc.vector.tensor_copy(q_sb[:, pr, sl], ps[:])
                else:
                    nc.scalar.activation(out=k_sb[:, pr, sl], in_=ps[:],
                                         func=AF.Copy)
        ps_qk.release()
        p_wqkv.release()
        p_x8.release()

        # bulk weight prefetch on the Activation HWDGE queue (separate rings
        # from the sync queue): fires right after the k drains, transfers
        # overlap the whole attention phase.
        p_wproj = tc.alloc_tile_pool(name="p_wproj", bufs=1)
        wproj_sb = p_wproj.tile([128, DC, D], F16)
        p_w2 = tc.alloc_tile_pool(name="p_w2", bufs=1)
        w2_sb = p_w2.tile([128, FC, D], F8)
        p_w1 = tc.alloc_tile_pool(name="p_w1", bufs=1)
        w1_sb = p_w1.tile([128, DC, HID], F8)
        with tc.tile_wait_until(0.028):
            nc.sync.dma_start(out=wproj_sb[:],
                              in_=wprojT[:, :].rearrange("(c p) d -> p c d", p=128))
            nc.sync.dma_start(out=x16_sb[:],
                              in_=xT16[:, :].rearrange("(c p) n -> p c n", p=128))
            nc.sync.dma_start(out=w1_sb[:],
                              in_=wfc1T[:, :].rearrange("(c p) h -> p c h", p=128))
            nc.sync.dma_start(out=w2_sb[:],
                              in_=wfc2T[:, :].rearrange("(f p) d -> p f d", p=128))

        # ------------- Phase 2: attention -------------
        p_ctx = tc.alloc_tile_pool(name="p_ctx", bufs=1)
        ctx_sb = p_ctx.tile([128, DC, N], F16)
        p_ae = tc.alloc_tile_pool(name="p_ae", bufs=6, side="right")
        p_craw = tc.alloc_tile_pool(name="p_craw", bufs=1, side="right")
        p_recb = tc.alloc_tile_pool(name="p_recb", bufs=2, side="right")
        ps_c = tc.alloc_tile_pool(name="ps_c", bufs=1, space="PSUM")

        craws = {}

        GROUPS = {2: (0, 6), 4: (6, 10), 5: (10, 12)}
        # per-group denominator gather tiles (base partition 0 to satisfy
        # the engine partition-quadrant rule); den rows DMA SBUF->SBUF
        # straight into their group row
        dgs = {}

        def grp_of(h):
            return 2 if h < 6 else (4 if h < 10 else 5)

        def den_group(g):
            """Batch-reciprocal denominators for a group of heads and
            normalize their context: one Ln + one Exp (same ACT table set
            as the softmax exp) instead of per-head reciprocals."""
            h0, h1 = GROUPS[g]
            nh = h1 - h0
            lng = stats.tile([6, N], F16, tag="lng", name="lng")
            nc.scalar.activation(out=lng[0:nh, :], in_=dgs[g][0:nh, :],
                                 func=AF.Ln)
            rec = stats.tile([6, N], F16, tag="rec", name="rec")
            nc.scalar.activation(out=rec[0:nh, :], in_=lng[0:nh, :],
                                 func=AF.Exp, scale=-1.0)
            rec_d = dscr.tile([6, N], F16, tag=f"recd{g}", name="rec_d")
            nc.sync.dma_start(out=rec_d[0:nh, :], in_=rec[0:nh, :])
            for i in range(nh):
                h = h0 + i
                prh, h01 = h // 2, h % 2
                half = h01 * 64
                recb = p_recb.tile([64, N], F16, tag="recb", name="recb")
                nc.sync.dma_start(
                    out=recb[:],
                    in_=rec_d[i:i + 1, :].to_broadcast([64, N]))
                nc.vector.tensor_mul(ctx_sb[half:half + 64, prh, :],
                                     craws[h][0:HD, :], recb[:])

        def den_group5_pe():
            """Last head pair's normalization without any DMA hops: Ln/Exp in
            place on each craw denominator row, reciprocal broadcast to 64
            partitions by a PE matmul into the (still idle) a/b-broadcast
            PSUM bank, context scaled by DVE reads straight from PSUM.
            Emitted as a proj hook so the broadcast matmuls queue behind the
            proj up-front groups instead of stalling the PE."""
            for h in (10, 11):
                row = craws[h][HD:HD + 1, :]
                nc.scalar.activation(out=row, in_=row, func=AF.Ln)
                nc.scalar.activation(out=row, in_=row, func=AF.Exp, scale=-1.0)
            abpt = ps_ab.tile([128, 2, 512], F32, tag="abp", name="recbP")
            for nb in range(NB):
                sl = slice(nb * 512, nb * 512 + 512)
                po = nb * 64
                for i, h in ((0, 10), (1, 11)):
                    nc.tensor.matmul(abpt[po:po + 64, i, :],
                                     oneshi[64:65, :],
                                     craws[h][HD:HD + 1, sl],
                                     start=True, stop=True)
                for i, h in ((0, 10), (1, 11)):
                    prh, h01 = h // 2, h % 2
                    half = h01 * 64
                    nc.vector.tensor_mul(ctx_sb[half:half + 64, prh, sl],
                                         craws[h][0:HD, sl],
                                         abpt[po:po + 64, i, :])

        cps = {}

        def emit_av(pr, mt, ae, ao):
            """attn@v for one (pr, mt) step; emitted two steps late so the
            PE's in-order stream stalls on neither the exp it depends on nor
            (at mt==0) the previous pair's craw drain of the accumulator
            banks. On the last token tile also fold in the per-pair
            epilogue: denominator rows are copied out first so the Ln/Exp
            reciprocal chain starts before the full context drain."""
            if mt == 0:
                for h01 in range(2):
                    cps[h01] = ps_c.tile([HD + 1, N], F32,
                                         tag=f"c{h01}", name=f"cps{h01}")
            for h01, at_t in ((0, ae), (1, ao)):
                h = 2 * pr + h01
                for nb in range(NB):
                    sl = slice(nb * 512, nb * 512 + 512)
                    nc.tensor.matmul(cps[h01][:, sl], v_sb[:, mt, h, :],
                                     at_t[:, sl],
                                     start=(mt == 0), stop=(mt == MT - 1))
            if mt == MT - 1:
                g = grp_of(2 * pr)
                if g != 5 and g not in dgs:
                    dgs[g] = stats.tile([6, N], F16, tag="dg", name=f"dg{g}")
                for h01 in range(2):
                    h = 2 * pr + h01
                    craw = p_craw.tile([HD + 1, N], F16,
                                       tag=f"cr{h % 6}", name="craw")
                    craws[h] = craw
                    nc.vector.tensor_copy(craw[:], cps[h01][:])
                    if g != 5:
                        nc.sync.dma_start(out=dgs[g][h - GROUPS[g][0]:
                                                     h - GROUPS[g][0] + 1, :],
                                          in_=craw[HD:HD + 1, :])
                if pr in (2, 4):
                    den_group(pr)

        pendings = []
        for pr in range(NPR):
            for mt in range(MT):
                msl = slice(mt * 128, mt * 128 + 128)
                pse = ps_s.tile([128, N], F32, tag="pse", name="pse")
                pso = ps_s.tile([128, N], F32, tag="pso", name="pso")
                for nb in range(NB):
                    sl = slice(nb * 512, nb * 512 + 512)
                    nc.tensor.matmul(pse[:, sl], k_sb[0:64, pr, msl],
                                     q_sb[0:64, pr, sl], start=True, stop=True)
                    nc.tensor.matmul(pso[:, sl], k_sb[64:128, pr, msl],
                                     q_sb[64:128, pr, sl], start=True, stop=True)
                ae = p_ae.tile([128, N], F16, tag="attnT", name="ae")
                ao = p_ae.tile([128, N], F16, tag="attnT", name="ao")
                nc.scalar.activation(out=ae[:], in_=pse[:], func=AF.Exp,
                                     scale=SOFT_SCALE)
                nc.scalar.activation(out=ao[:], in_=pso[:], func=AF.Exp,
                                     scale=SOFT_SCALE)
                pendings.append((pr, mt, ae, ao))
                if len(pendings) > 2:
                    emit_av(*pendings.pop(0))
        for p in pendings:
            emit_av(*p)
        ps_c.release()
        ps_s.release()
        p_recb.release()
        p_craw.release()
        p_ae.release()
        p_v.release()
        p_qk.release()

        # ------------- Phase 3+4: proj + LN1 + MLP + LN2 -------------
        p_r1 = tc.alloc_tile_pool(name="p_r1", bufs=1)
        r1_sb = p_r1.tile([128, DC, N], F16)
        p_y2 = tc.alloc_tile_pool(name="p_y2", bufs=1)
        p_h = tc.alloc_tile_pool(name="p_h", bufs=1)
        p_y16 = tc.alloc_tile_pool(name="p_y16", bufs=3)
        y2_sb = p_y2.tile([128, DC, N], F16)
        h_sb = p_h.tile([128, FC, N], F8)
        # ps_mm first so its four banks alias the score-psum banks (released
        # at the last exp read, ~2us before the attn@v accumulators drain) --
        # the proj up-front matmuls start that much earlier
        ps_mm = tc.alloc_tile_pool(name="ps_mm", bufs=4, space="PSUM")
        ps_ln = tc.alloc_tile_pool(name="ps_ln", bufs=1, space="PSUM")
        ps_ab = tc.alloc_tile_pool(name="ps_ab", bufs=1, space="PSUM")

        def ln_accum(st, src_sb, et, sl, first, last, sq_eng=None):
            """Fold chunk et of the pre-norm tensor into the LN sums. The s1/s2
            PSUM rows are allocated lazily on first call so the tag-rotating
            WAR dep lands after the previous user's finish chain is emitted.
            The square runs on GPSIMD (otherwise idle) for some chunks."""
            w = sl.stop - sl.start
            if "s1" not in st:
                st["s1"] = ps_ln.tile([1, 512], F32, tag="s1", name="s1")
                st["s2"] = ps_ln.tile([1, 512], F32, tag="s2", name="s2")
            sqt = p_u.tile([128, 512], F16, tag="sqt", name="sqt")
            eng = sq_eng or nc.vector
            eng.tensor_mul(sqt[:, 0:w], src_sb[:, et, sl], src_sb[:, et, sl])
            nc.tensor.matmul(st["s1"][:, 0:w], m16[:], src_sb[:, et, sl],
                             start=first, stop=last)
            nc.tensor.matmul(st["s2"][:, 0:w], p16[:], sqt[:, 0:w],
                             start=first, stop=last)

        def ln_finish(st, src_sb, sl, goff, boff, out_sb, out_f16=None,
                      out_dram=None, aff_eng=None, cast_eng=None,
                      split_out=False, out_f8=None):
            """Stats chain + affine for token slice sl. rsqrt(var+eps) is
            Exp(-0.5*Ln(.)) so everything stays in the nl-exp ACT set.
            Scalar chain is 3 ops (Square, Ln, Exp); the -1/D mean scale is
            folded into the negrow lhsT of the b-broadcast matmul. aff_eng
            picks the engine for the big affine ops (gpsimd for the
            non-critical LN2 half so the vector engine stays clear for the
            tail); cast_eng=scalar moves the a/b PSUM->SBUF cast off the
            vector engine; split_out pipelines the affine with the output
            DMA in two 3-chunk halves."""
            w = sl.stop - sl.start
            s1, s2 = st["s1"][:, 0:w], st["s2"][:, 0:w]
            ae_ = aff_eng or nc.vector
            t4 = stats.tile([1, 512], F32, tag="t4", name="t4")[:, 0:w]
            t2 = stats.tile([1, 512], F32, tag="t2", name="t2")[:, 0:w]
            t31f = stats.tile([1, 2, 512], F32R, tag="t31", name="t31")
            t3, t1 = t31f[:, 0, 0:w], t31f[:, 1, 0:w]  # a=1/std, b=-mu/std
            nc.scalar.activation(out=t4, in_=s1, func=AF.Square)  # mean^2
            nc.vector.tensor_sub(t2, s2, t4)                      # var
            nc.scalar.activation(out=t4, in_=t2, func=AF.Ln, bias=eps_t[:])
            nc.scalar.activation(out=t3, in_=t4, func=AF.Exp,
                                 scale=-0.5)                     # rsqrt(var+eps)
            nc.vector.tensor_mul(t1, s1, t3.bitcast(F32))
            abp = ps_ab.tile([128, 2, 512], F32, tag="abp", name="abp")
            if w <= 256:
                # one broadcast matmul for both a and b rows: out packs
                # [128, 2, w] inside a single PSUM bank
                abpv = abp[:, 0, 0:2 * w].rearrange("p (t w) -> p t w", t=2)
                nc.tensor.matmul(abpv, onesrow[:], t31f[:, :, 0:w],
                                 start=True, stop=True)
            else:
                abpv = abp[:, :, 0:w]
                nc.tensor.matmul(abp[:, 0, 0:w], onesrow[:], t3,
                                 start=True, stop=True)
                nc.tensor.matmul(abp[:, 1, 0:w], onesrow[:], t1,
                                 start=True, stop=True)
            abps = stats.tile([128, 2, 512], F16, tag=f"abps{sl.start}",
                              name="abps")
            if cast_eng is nc.scalar:
                nc.scalar.activation(out=abps[:, :, 0:w], in_=abpv,
                                     func=AF.Copy)
            else:
                (cast_eng or nc.vector).tensor_copy(abps[:, :, 0:w], abpv)

            def dma_half(half):
                cs = slice(3 * half, 3 * half + 3)
                nc.sync.dma_start(
                    out=out_dram[384 * half:384 * half + 384, sl].rearrange(
                        "(c p) n -> p c n", p=128),
                    in_=out_f16[:, cs, 0:w])

            if trivial_affine:
                # gamma==1, beta==0: out = src*a + b for all 6 chunks in
                # fused DVE ops using zero-stride broadcast of the per-token
                # scale/shift rows
                uall = p_sq.tile([128, DC, 512], F16, tag=f"uall{sl.start}",
                                 name="uall")
                if split_out:
                    pieces = [slice(0, 3), slice(3, 6)]
                elif out_f8 is not None:
                    # 2-chunk pieces so the fc1 DR pairs can start on the
                    # first chunk pair before the whole affine is done
                    pieces = [slice(0, 2), slice(2, 4), slice(4, 6)]
                else:
                    pieces = [slice(0, DC)]
                for pi, cs in enumerate(pieces):
                    nch = cs.stop - cs.start
                    ab0 = abps[:, 0:1, 0:w].to_broadcast([128, nch, w])
                    ab1 = abps[:, 1:2, 0:w].to_broadcast([128, nch, w])
                    ae_.tensor_mul(uall[:, cs, 0:w], src_sb[:, cs, sl], ab0)
                    if out_f16 is not None:
                        ae_.tensor_add(out_f16[:, cs, 0:w], uall[:, cs, 0:w], ab1)
                        if out_dram is not None and split_out:
                            dma_half(pi)
                    else:
                        ae_.tensor_add(out_sb[:, cs, sl], uall[:, cs, 0:w], ab1)
                        if out_f8 is not None:
                            ae_.tensor_scalar_mul(out_f8[:, cs, sl],
                                                  in0=out_sb[:, cs, sl],
                                                  scalar1=XS)
            else:
                for c in range(DC):
                    u = p_u.tile([128, 512], F16, tag="u", name="u")
                    ae_.tensor_mul(u[:, 0:w], src_sb[:, c, sl], abps[:, 0, 0:w])
                    ae_.tensor_add(u[:, 0:w], u[:, 0:w], abps[:, 1, 0:w])
                    dst = out_f16[:, c, 0:w] if out_f16 is not None else out_sb[:, c, sl]
                    ae_.tensor_scalar(out=dst, in0=u[:, 0:w],
                                      scalar1=consts_sb[:, goff + c:goff + c + 1],
                                      scalar2=consts_sb[:, boff + c:boff + c + 1],
                                      op0=OP.mult, op1=OP.add)
                    if out_f8 is not None:
                        ae_.tensor_scalar_mul(out_f8[:, c, sl],
                                              in0=out_sb[:, c, sl], scalar1=XS)
                    if out_dram is not None and split_out and c in (2, 5):
                        dma_half(c // 3)
            if out_dram is not None and not split_out:
                nc.sync.dma_start(
                    out=out_dram[:, sl].rearrange("(c p) n -> p c n", p=128),
                    in_=out_f16[:, :, 0:w])

        def proj_block(nb, st, pe_extras, defer_accums, fin_hook=None):
            """proj matmuls for token half nb. ctx chunks 0..4 are contracted
            for four et groups up front (the den-gated chunk 5 last) so the
            PE has queued work while the last denominator chain completes.
            pe_extras: deferred PE ops (prev half's tail accums) emitted
            after the up-front groups. Returns this half's deferred accum
            lambdas (all 6 when defer_accums, else the last 4)."""
            sl = slice(nb * 512, nb * 512 + 512)

            def drain(et, ps):
                t = p_u.tile([128, 512], F16, tag="pt", name="pt")
                nc.scalar.activation(out=t[:], in_=ps[:], func=AF.Identity,
                                     bias=consts_sb[:, CBPJ + et:CBPJ + et + 1],
                                     scale=1.0)
                nc.vector.tensor_add(r1_sb[:, et, sl], t[:], x16_sb[:, et, sl])

            pss = {}
            for et in range(4):
                pss[et] = ps_mm.tile([128, 512], F32, tag="mm", name="pspj")
            for c in range(5):
                for et in range(4):
                    nc.tensor.matmul(pss[et][:],
                                     wproj_sb[:, c, et * 128:(et + 1) * 128],
                                     ctx_sb[:, c, sl],
                                     start=(c == 0), stop=False)
            for fn in pe_extras:
                fn()
            for et in range(4):
                nc.tensor.matmul(pss[et][:],
                                 wproj_sb[:, 5, et * 128:(et + 1) * 128],
                                 ctx_sb[:, 5, sl], start=False, stop=True)
                drain(et, pss[et])
            for et in (4, 5):
                ps = ps_mm.tile([128, 512], F32, tag="mm", name="pspj")
                for c in range(DC):
                    nc.tensor.matmul(ps[:], wproj_sb[:, c, et * 128:(et + 1) * 128],
                                     ctx_sb[:, c, sl],
                                     start=(c == 0), stop=(c == DC - 1))
                drain(et, ps)
                if not defer_accums:
                    ln_accum(st, r1_sb, et - 4, sl, et - 4 == 0, False)
            if fin_hook is not None:
                fin_hook()
            first = 2 if not defer_accums else 0
            return [lambda e=e: ln_accum(st, r1_sb, e, sl, e == 0, e == DC - 1)
                    for e in range(first, DC)]

        def fc1_block(nb, extras):
            sl = slice(nb * 512, nb * 512 + 512)
            for ftg in range(FC):
                ps = ps_mm.tile([128, 512], F32, tag="mm", name="psf1")
                for j in range(DC // 2):
                    nc.tensor.matmul(ps[:],
                                     w1_sb[:, 2 * j:2 * j + 2,
                                           ftg * 128:(ftg + 1) * 128],
                                     x18_sb[:, 2 * j:2 * j + 2, sl],
                                     start=(j == 0), stop=(j == DC // 2 - 1),
                                     perf_mode=DR)
                nc.scalar.activation(out=h_sb[:, ftg, sl], in_=ps[:], func=AF.Gelu,
                                     bias=consts_sb[:, CBF1 + ftg:CBF1 + ftg + 1],
                                     scale=1.0 / (XS * WS))
                if ftg in extras:
                    extras[ftg]()

        def fc2_group(sl, et):
            w = sl.stop - sl.start
            ps = ps_mm.tile([128, 512], F32, tag="mm", name="psf2")
            for j in range(FC // 2):
                nc.tensor.matmul(ps[:, 0:w],
                                 w2_sb[:, 2 * j:2 * j + 2, et * 128:(et + 1) * 128],
                                 h_sb[:, 2 * j:2 * j + 2, sl],
                                 start=(j == 0), stop=(j == FC // 2 - 1),
                                 perf_mode=DR)
            t = p_u.tile([128, 512], F16, tag="ft", name="ft")
            nc.scalar.activation(out=t[:, 0:w], in_=ps[:, 0:w], func=AF.Identity,
                                 bias=consts_sb[:, CBF2 + et:CBF2 + et + 1],
                                 scale=1.0 / WS)
            nc.vector.tensor_add(y2_sb[:, et, sl], t[:, 0:w], x1_sb[:, et, sl])

        stA, stB, stC = {}, {}, {}
        sl0, sl1 = slice(0, 512), slice(512, 1024)
        accA = proj_block(0, stA, [den_group5_pe], defer_accums=False)
        # accums for et 2..5 of nb0 interleave into nb1's up-front groups and
        # fin(nb0) is emitted at nb1's block end; nb1 defers all its accums
        # into fc1(nb0) so the s1/s2 bank WAR lands after fin(nb0)'s reads,
        # and fin(nb1) lands mid-fc1(nb0) so x1(nb1) is ready well before
        # fc1(nb1) while its broadcast matmuls never stall the PE stream.
        accB = proj_block(1, stB, accA, defer_accums=True,
                          fin_hook=lambda: ln_finish(stA, r1_sb, sl0, CG1, CB1,
                                                     x1_sb, out_f8=x18_sb))

        def pair(fns):
            return lambda: [fn() for fn in fns]

        fc1_extras = {i: pair(accB[2 * i:2 * i + 2]) for i in range(3)}
        fc1_extras[15] = lambda: ln_finish(stB, r1_sb, sl1, CG1, CB1, x1_sb,
                                           out_f8=x18_sb)
        fc1_block(0, fc1_extras)
        fc1_block(1, {})

        # fc2 nb0: full 512 block, internal defer-by-2 accums for et 0..3
        for et in range(DC):
            fc2_group(sl0, et)
            if et >= 2:
                ln_accum(stC, y2_sb, et - 2, sl0, et - 2 == 0, False)
        # fc2 nb1 runs as two 256-token quarters so quarter 0's LN2 finish
        # overlaps quarter 1's matmuls and only the last quarter's stats
        # chain is exposed at the very end.
        stD0, stD1 = {}, {}
        slq0, slq1 = slice(512, 768), slice(768, 1024)
        y16a = p_y16.tile([128, DC, 512], F16, tag="y16", name="y16a")
        for et in range(DC):
            fc2_group(slq0, et)
            if et == 0:
                ln_accum(stC, y2_sb, 4, sl0, False, False)
            elif et == 1:
                ln_accum(stC, y2_sb, 5, sl0, False, True)
            elif et == 2:
                ln_finish(stC, y2_sb, sl0, CG2, CB2, None, out_f16=y16a,
                          out_dram=yT16, cast_eng=nc.scalar)
            else:
                ln_accum(stD0, y2_sb, et - 3, slq0, et - 3 == 0, False,
                         sq_eng=nc.vector)
        y16b = p_y16.tile([128, DC, 512], F16, tag="y16", name="y16b")
        for et in range(DC):
            fc2_group(slq1, et)
            if et < 3:
                ln_accum(stD0, y2_sb, et + 3, slq0, False, et == 2,
                         sq_eng=nc.vector)
            elif et == 3:
                ln_finish(stD0, y2_sb, slq0, CG2, CB2, None, out_f16=y16b,
                          out_dram=yT16, cast_eng=nc.scalar)
            else:
                ln_accum(stD1, y2_sb, et - 4, slq1, et - 4 == 0, False,
                         sq_eng=nc.vector)
        for e in (2, 3, 4, 5):
            ln_accum(stD1, y2_sb, e, slq1, False, e == DC - 1, sq_eng=nc.vector)
        y16c = p_y16.tile([128, DC, 512], F16, tag="y16", name="y16c")
        ln_finish(stD1, y2_sb, slq1, CG2, CB2, None, out_f16=y16c,
                  out_dram=yT16, cast_eng=nc.scalar, split_out=True)
        ps_ab.release()
        ps_ln.release()
        ps_mm.release()
        dscr.release()
        p_y16.release()
        p_h.release()
        p_y2.release()
        p_r1.release()
        p_ctx.release()
        p_w1.release()
        p_w2.release()
        p_wproj.release()
        p_x16.release()
        p_u.release()
        p_sq.release()
        p_x1.release()
        stats.release()
        const.release()
    return nc


_NC_CACHE = {}


def _get_nc(trivial_affine=False):
    nc = _NC_CACHE.get(trivial_affine)
    if nc is None:
        nc = _build(trivial_affine)
        _split_excess_waits(nc)
        _NC_CACHE[trivial_affine] = nc
    return nc


def kernel(x, w_qkv, w_proj, b_proj, w_fc1, b_fc1, w_fc2, b_fc2,
           gamma1, beta1, gamma2, beta2):
    global LAST_RESULT
    x = np.asarray(x, dtype=np.float32)
    w_qkv = np.asarray(w_qkv, dtype=np.float32)
    w_proj = np.asarray(w_proj, dtype=np.float32)
    b_proj = np.asarray(b_proj, dtype=np.float32)
    w_fc1 = np.asarray(w_fc1, dtype=np.float32)
    b_fc1 = np.asarray(b_fc1, dtype=np.float32)
    w_fc2 = np.asarray(w_fc2, dtype=np.float32)
    b_fc2 = np.asarray(b_fc2, dtype=np.float32)
    gamma1 = np.asarray(gamma1, dtype=np.float32)
    beta1 = np.asarray(beta1, dtype=np.float32)
    gamma2 = np.asarray(gamma2, dtype=np.float32)
    beta2 = np.asarray(beta2, dtype=np.float32)

    F8NP = ml_dtypes.float8_e4m3
    wqkv_scaled = w_qkv.copy()
    wqkv_scaled[:D] *= HD ** -0.5                  # fold attention scale into Q
    wqkvT = np.ascontiguousarray((wqkv_scaled.T * WS).astype(F8NP))
    wprojT = np.ascontiguousarray(w_proj.T.astype(np.float16))
    wfc1T = np.ascontiguousarray((w_fc1.T * WS).astype(F8NP))
    wfc2T = np.ascontiguousarray((w_fc2.T * WS).astype(F8NP))

    def cols(v, nchunks):
        return np.ascontiguousarray(v.reshape(nchunks, 128).T)

    constsC = np.ascontiguousarray(np.hstack([
        cols(b_proj, DC), cols(b_fc1, FC), cols(b_fc2, DC),
        cols(gamma1, DC), cols(beta1, DC), cols(gamma2, DC), cols(beta2, DC),
    ]).astype(np.float32))

    shared = {
        "wqkvT": wqkvT, "wprojT": wprojT, "wfc1T": wfc1T, "wfc2T": wfc2T,
        "constsC": constsC,
    }
    in_maps = []
    for b in range(NCORES):
        m = dict(shared)
        xt = np.ascontiguousarray(x[b].T)
        m["xT16"] = xt.astype(np.float16)
        m["xT8"] = (xt * XS).astype(F8NP)
        in_maps.append(m)

    trivial = (np.all(gamma1 == 1.0) and np.all(beta1 == 0.0)
               and np.all(gamma2 == 1.0) and np.all(beta2 == 0.0))
    nc = _get_nc(trivial_affine=bool(trivial))
    LAST_RESULT = run_bass_kernel_spmd(nc, in_maps, list(range(NCORES)))
    out = np.stack([np.ascontiguousarray(LAST_RESULT.results[b]["yT16"].T)
                    for b in range(NCORES)])
    return out.astype(np.float32)


# revision 66
# speedup vs baseline: 1.1464x; 1.0927x over previous
"""Trainium2 Bass kernel for a prenorm transformer Block (B=8, N=1024, D=768,
12 heads, MLP hidden 3072), data-parallel over batch across 8 NeuronCores.

v3. Engine/queue-level restructure of v2:

  - Dual DMA queues: bulk weight prefetch (wproj/x16/w1/w2, 9.4MB) triggers on
    the Activation HWDGE queue right after the qk phase, so the Sync queue and
    its rings stay clear for the latency-critical small DMAs (softmax
    denominator gathers/broadcasts) during attention. v2 issued w1/w2 on the
    sync queue *after* the attention stream, so their transfers only started
    near attention end and the fc1 phase raced weight arrival.
  - Scores matmuls contract 64 real K rows per head (even head on PE rows
    0:63, odd on rows 64:127 via inferred tile_position) -- no zero-padded K
    planes, no 5us memset, k-drain is one copy instead of two.
  - Softmax exp merged: one [128, 2x1024] Exp per (pair, token-tile) instead
    of two [128,1024] (saves the ~470ns fixed cost per ACT op; the exp is the
    scalar-engine bottleneck of the attention phase). q/k are drained
    unscaled; the fp8 dequant scale is folded into the Exp's scale operand.
  - Denominator rows gather SBUF->SBUF into adjacent partitions (no DRAM
    round trip before the Ln/Exp reciprocal).
  - proj contracts ctx chunks 0..4 for four output chunks first (4 concurrent
    PSUM groups) and the den-gated chunk 5 last, so the PE has ~4.3us of work
    queued while the last head pair's denominator chain completes.
  - LN finishes are emitted *after* the next matmul block so their a/b
    broadcast matmuls never stall the in-order PE stream (fin1 after fc1(nb0),
    fin2(nb0) two et-groups into fc2(nb1)); LN stats chain loses two scalar
    ops (mean scale folded into a -1/D ones-row on the b-broadcast matmul,
    E[x^2] scale moved to DVE).
  - Output is fp16 (converted to f32 host-side), written with one merged
    rearranged DMA per token half.
"""
import sys
import types

sys.path.insert(0, "/opt/trn_rl_repo")

try:
    import antenv.axon_hooks  # noqa: F401
except Exception:
    try:
        import antenv

        _hooks = types.ModuleType("antenv.axon_hooks")
        _hooks._hook = None

        def _set_hook(h):
            _hooks._hook = h

        def _get_hook():
            return _hooks._hook

        _hooks.set_axon_ntff_profile_hook = _set_hook
        _hooks.get_axon_ntff_profile_hook = _get_hook
        sys.modules["antenv.axon_hooks"] = _hooks
        antenv.axon_hooks = _hooks
    except Exception:
        pass

import ml_dtypes
import numpy as np

import concourse.bass as bass
import concourse.tile as tile
from concourse import mybir
from concourse.bass_utils import run_bass_kernel_spmd

F32R = mybir.dt.float32r
F32 = mybir.dt.float32
F16 = mybir.dt.float16
F8 = mybir.dt.float8e4
DR = mybir.MatmulPerfMode.DoubleRow
AF = mybir.ActivationFunctionType
OP = mybir.AluOpType
XS, WS = 16.0, 256.0                 # fp8 scales: x, weights

NCORES = 8
D, HEADS, HID, N = 768, 12, 3072, 1024
HD = D // HEADS                  # 64 head dim
DC = D // 128                    # 6 feature chunks
FC = HID // 128                  # 24 hidden chunks
NB = N // 512                    # 2 moving-dim blocks
MT = N // 128                    # 8 token tiles
NPR = HEADS // 2                 # 6 head pairs
EPS = 1e-6
SOFT_SCALE = 1.0 / (XS * WS) ** 2

# packed per-feature constants: column offsets in constsC
CBPJ, CBF1, CBF2, CG1, CB1, CG2, CB2 = 0, 6, 30, 36, 42, 48, 54

LAST_RESULT = None


# The walrus build in this container rejects instructions carrying more than
# a couple of sync waits; hoist excess waits onto standalone EventSemaphore
# carriers on the same engine (semantically identical).
_MM_OPS = ("Matmult", "Ldweights")


def _split_excess_waits(nc, default_limit=1, matmul_limit=0):
    counter = 0
    for f in nc.m.functions:
        for bb in f.blocks:
            new_insts = []
            for inst in bb.instructions:
                si = inst.sync_info
                waits = list(si.on_wait) if si and si.on_wait else []
                limit = matmul_limit if inst.opcode in _MM_OPS else default_limit
                if len(waits) > limit:
                    keep, move = waits[:limit], waits[limit:]
                    for w in move:
                        counter += 1
                        ev = mybir.InstEventSemaphore(
                            name=f"I-waitsplit-{counter}",
                            engine=inst.engine,
                            sync_info=mybir.SyncInfo(on_wait=[w], on_update=[]),
                        )
                        nc.register_instruction(ev, overwrite=True)
                        new_insts.append(ev)
                    inst.sync_info = mybir.SyncInfo(
                        on_wait=keep, on_update=list(si.on_update) if si else []
                    )
                new_insts.append(inst)
            bb.instructions = new_insts
    return counter


def _build(trivial_affine=False):
    nc = bass.Bass()

    xT16 = nc.dram_tensor("xT16", [D, N], F16, kind="ExternalInput")
    xT8 = nc.dram_tensor("xT8", [D, N], F8, kind="ExternalInput")
    wqkvT = nc.dram_tensor("wqkvT", [D, 3 * D], F8, kind="ExternalInput")
    wprojT = nc.dram_tensor("wprojT", [D, D], F16, kind="ExternalInput")
    wfc1T = nc.dram_tensor("wfc1T", [D, HID], F8, kind="ExternalInput")
    wfc2T = nc.dram_tensor("wfc2T", [HID, D], F8, kind="ExternalInput")
    constsC = nc.dram_tensor("constsC", [128, 60], F32, kind="ExternalInput")
    yT16 = nc.dram_tensor("yT16", [D, N], F16, kind="ExternalOutput")

    with tile.TileContext(nc) as tc:
        # ---- long-lived left-side pools (pushed first, released last) ----
        const = tc.alloc_tile_pool(name="const", bufs=1)
        stats = tc.alloc_tile_pool(name="stats", bufs=1)
        p_x1 = tc.alloc_tile_pool(name="p_x1", bufs=1)
        p_sq = tc.alloc_tile_pool(name="p_sq", bufs=1)
        p_u = tc.alloc_tile_pool(name="p_u", bufs=2)
        dscr = tc.alloc_tile_pool(name="dscr", bufs=1, space="DRAM")

        onesrow = const.tile([1, 128], F32R)
        nc.vector.tensor_copy(onesrow[:], nc.const_aps.tensor(1.0, (1, 128)))
        # LN sum lhsT columns with the 1/D mean scale folded in: s1 = -mean,
        # s2 = E[x^2], so the finish chain needs no separate scale ops and
        # both a/b broadcasts use the same onesrow lhsT.
        m16 = const.tile([128, 1], F16)
        nc.vector.memset(m16[:], -1.0 / D)
        p16 = const.tile([128, 1], F16)
        nc.vector.memset(p16[:], 1.0 / D)
        # ones row on partitions 64(+) for the PE-broadcast of the last
        # head pair's softmax reciprocal (contraction row = craw's den row)
        oneshi = const.tile([128, 64], F16)
        nc.vector.memset(oneshi[64:66, :], 1.0)
        eps_t = const.tile([1, 1], F32)
        nc.vector.memset(eps_t[:], EPS)
        consts_sb = const.tile([128, 60], F32)
        x1_sb = p_x1.tile([128, DC, N], F16)
        x18_sb = p_x1.tile([128, DC, N], F8)   # fp8 copy of x1 for the fc1 DR
        warm = stats.tile([1, 8], F32, tag="warm", name="warm")
        nc.vector.memset(warm[:], 1.0)
        nc.scalar.activation(out=warm[:], in_=warm[:], func=AF.Exp)

        # ------------- Phase 1: QKV projections -------------
        # startup DMAs on the sync queue in exact consumption order: v-column
        # weights + x8 first halves interleaved (the v matmuls are the
        # kernel's first tensor work), then x8 second halves, q cols, k cols.
        p_x16 = tc.alloc_tile_pool(name="p_x16", bufs=1)
        x16_sb = p_x16.tile([128, DC, N], F16)
        p_x8 = tc.alloc_tile_pool(name="p_x8", bufs=1)
        x8_sb = p_x8.tile([128, DC, N], F8)
        p_wqkv = tc.alloc_tile_pool(name="p_wqkv", bufs=1)
        wqkv_sb = p_wqkv.tile([128, DC, 3 * D], F8)

        nc.sync.dma_start(out=consts_sb[:], in_=constsC[:, :])
        for cp in range(DC // 2):
            for c in (2 * cp, 2 * cp + 1):
                nc.sync.dma_start(out=wqkv_sb[:, c, 2 * D:3 * D],
                                  in_=wqkvT[c * 128:(c + 1) * 128, 2 * D:3 * D])
            for c in (2 * cp, 2 * cp + 1):
                nc.sync.dma_start(out=x8_sb[:, c, 0:512],
                                  in_=xT8[c * 128:(c + 1) * 128, 0:512])
        for c in range(DC):
            nc.sync.dma_start(out=x8_sb[:, c, 512:N],
                              in_=xT8[c * 128:(c + 1) * 128, 512:N])
        for c in range(DC):
            nc.sync.dma_start(out=wqkv_sb[:, c, 0:D],
                              in_=wqkvT[c * 128:(c + 1) * 128, 0:D])
        for c in range(DC):
            nc.sync.dma_start(out=wqkv_sb[:, c, D:2 * D],
                              in_=wqkvT[c * 128:(c + 1) * 128, D:2 * D])

        p_qk = tc.alloc_tile_pool(name="p_qk", bufs=1, side="right")
        p_v = tc.alloc_tile_pool(name="p_v", bufs=1, side="right")
        q_sb = p_qk.tile([128, DC, N], F16)
        k_sb = p_qk.tile([128, DC, N], F16)
        v_sb = p_v.tile([128, MT, HEADS, HD + 1], F16)
        nc.vector.memset(v_sb[:, :, :, HD:HD + 1], 1.0)

        # v in direct layout: [token (partitions), v-dim]; drains alternate
        # vector/scalar (both idle here)
        ps_v = tc.alloc_tile_pool(name="ps_v", bufs=2, space="PSUM")
        for mt in range(MT):
            msl = slice(mt * 128, mt * 128 + 128)
            ps = ps_v.tile([128, D], F32, tag="v", name="psv")
            for j in range(DC // 2):
                nc.tensor.matmul(ps[:, 0:512], x8_sb[:, 2 * j:2 * j + 2, msl],
                                 wqkv_sb[:, 2 * j:2 * j + 2, 2 * D:2 * D + 512],
                                 start=(j == 0), stop=(j == DC // 2 - 1),
                                 perf_mode=DR)
                nc.tensor.matmul(ps[:, 512:768], x8_sb[:, 2 * j:2 * j + 2, msl],
                                 wqkv_sb[:, 2 * j:2 * j + 2, 2 * D + 512:3 * D],
                                 start=(j == 0), stop=(j == DC // 2 - 1),
                                 perf_mode=DR)
            vout = v_sb[:, mt, :, 0:HD]
            vin = ps[:].rearrange("p (h d) -> p h d", h=HEADS)
            if mt % 2 == 0:
                nc.vector.tensor_scalar_mul(vout, in0=vin, scalar1=1.0 / (XS * WS))
            else:
                nc.scalar.activation(out=vout, in_=vin, func=AF.Copy,
                                     scale=1.0 / (XS * WS))
        ps_v.release()

        # scores psum allocated BEFORE the qk pool so the first score
        # matmuls don't wait for the whole qk-phase psum to drain
        ps_s = tc.alloc_tile_pool(name="ps_s", bufs=1, space="PSUM")
        # q,k transposed: [qkv-row tile (partitions), tokens]; drains are
        # plain copies (fp8 dequant scale folded into the softmax Exp)
        ps_qk = tc.alloc_tile_pool(name="ps_qk", bufs=4, space="PSUM")
        for jt in [x for p in range(DC) for x in (p, DC + p)]:
            pr = jt % DC
            col0 = jt * 128
            for nb in range(NB):
                sl = slice(nb * 512, nb * 512 + 512)
                ps = ps_qk.tile([128, 512], F32, tag="qk", name="psqk")
                for j in range(DC // 2):
                    nc.tensor.matmul(ps[:], wqkv_sb[:, 2 * j:2 * j + 2, col0:col0 + 128],
                                     x8_sb[:, 2 * j:2 * j + 2, sl],
                                     start=(j == 0), stop=(j == DC // 2 - 1),
                                     perf_mode=DR)
                if jt < DC:
                    nc.vector.tensor_copy(q_sb[:, pr, sl], ps[:])
                else:
                    nc.scalar.activation(out=k_sb[:, pr, sl], in_=ps[:],
                                         func=AF.Copy)
        ps_qk.release()
        p_wqkv.release()
        p_x8.release()

        # bulk weight prefetch on the Activation HWDGE queue (separate rings
        # from the sync queue): fires right after the k drains, transfers
        # overlap the whole attention phase.
        p_wproj = tc.alloc_tile_pool(name="p_wproj", bufs=1)
        wproj_sb = p_wproj.tile([128, DC, D], F16)
        p_w2 = tc.alloc_tile_pool(name="p_w2", bufs=1)
        w2_sb = p_w2.tile([128, FC, D], F8)
        p_w1 = tc.alloc_tile_pool(name="p_w1", bufs=1)
        w1_sb = p_w1.tile([128, DC, HID], F8)
        with tc.tile_wait_until(0.028):
            nc.sync.dma_start(out=wproj_sb[:],
                              in_=wprojT[:, :].rearrange("(c p) d -> p c d", p=128))
            nc.sync.dma_start(out=x16_sb[:],
                              in_=xT16[:, :].rearrange("(c p) n -> p c n", p=128))
            nc.sync.dma_start(out=w1_sb[:],
                              in_=wfc1T[:, :].rearrange("(c p) h -> p c h", p=128))
            nc.sync.dma_start(out=w2_sb[:],
                              in_=wfc2T[:, :].rearrange("(f p) d -> p f d", p=128))

        # ------------- Phase 2: attention -------------
        p_ctx = tc.alloc_tile_pool(name="p_ctx", bufs=1)
        ctx_sb = p_ctx.tile([128, DC, N], F16)
        p_ae = tc.alloc_tile_pool(name="p_ae", bufs=6, side="right")
        p_craw = tc.alloc_tile_pool(name="p_craw", bufs=1, side="right")
        p_recb = tc.alloc_tile_pool(name="p_recb", bufs=2, side="right")
        ps_c = tc.alloc_tile_pool(name="ps_c", bufs=1, space="PSUM")

        craws = {}

        GROUPS = {2: (0, 6), 4: (6, 10), 5: (10, 12)}
        # per-group denominator gather tiles (base partition 0 to satisfy
        # the engine partition-quadrant rule); den rows DMA SBUF->SBUF
        # straight into their group row
        dgs = {}

        def grp_of(h):
            return 2 if h < 6 else (4 if h < 10 else 5)

        def den_group(g):
            """Batch-reciprocal denominators for a group of heads and
            normalize their context: one Ln + one Exp (same ACT table set
            as the softmax exp) instead of per-head reciprocals."""
            h0, h1 = GROUPS[g]
            nh = h1 - h0
            lng = stats.tile([6, N], F16, tag="lng", name="lng")
            nc.scalar.activation(out=lng[0:nh, :], in_=dgs[g][0:nh, :],
                                 func=AF.Ln)
            rec = stats.tile([6, N], F16, tag="rec", name="rec")
            nc.scalar.activation(out=rec[0:nh, :], in_=lng[0:nh, :],
                                 func=AF.Exp, scale=-1.0)
            rec_d = dscr.tile([6, N], F16, tag=f"recd{g}", name="rec_d")
            nc.sync.dma_start(out=rec_d[0:nh, :], in_=rec[0:nh, :])
            for i in range(nh):
                h = h0 + i
                prh, h01 = h // 2, h % 2
                half = h01 * 64
                recb = p_recb.tile([64, N], F16, tag="recb", name="recb")
                nc.sync.dma_start(
                    out=recb[:],
                    in_=rec_d[i:i + 1, :].to_broadcast([64, N]))
                nc.vector.tensor_mul(ctx_sb[half:half + 64, prh, :],
                                     craws[h][0:HD, :], recb[:])

        def den_group5_pe():
            """Last head pair's normalization without any DMA hops: Ln/Exp in
            place on each craw denominator row, reciprocal broadcast to 64
            partitions by a PE matmul into the (still idle) a/b-broadcast
            PSUM bank, context scaled by DVE reads straight from PSUM.
            Emitted as a proj hook so the broadcast matmuls queue behind the
            proj up-front groups instead of stalling the PE."""
            for h in (10, 11):
                row = craws[h][HD:HD + 1, :]
                nc.scalar.activation(out=row, in_=row, func=AF.Ln)
                nc.scalar.activation(out=row, in_=row, func=AF.Exp, scale=-1.0)
            abpt = ps_ab.tile([128, 2, 512], F32, tag="abp", name="recbP")
            for nb in range(NB):
                sl = slice(nb * 512, nb * 512 + 512)
                po = nb * 64
                for i, h in ((0, 10), (1, 11)):
                    nc.tensor.matmul(abpt[po:po + 64, i, :],
                                     oneshi[64:65, :],
                                     craws[h][HD:HD + 1, sl],
                                     start=True, stop=True)
                for i, h in ((0, 10), (1, 11)):
                    prh, h01 = h // 2, h % 2
                    half = h01 * 64
                    nc.vector.tensor_mul(ctx_sb[half:half + 64, prh, sl],
                                         craws[h][0:HD, sl],
                                         abpt[po:po + 64, i, :])

        cps = {}

        def emit_av(pr, mt, ae, ao):
            """attn@v for one (pr, mt) step; emitted two steps late so the
            PE's in-order stream stalls on neither the exp it depends on nor
            (at mt==0) the previous pair's craw drain of the accumulator
            banks. On the last token tile also fold in the per-pair
            epilogue: denominator rows are copied out first so the Ln/Exp
            reciprocal chain starts before the full context drain."""
            if mt == 0:
                for h01 in range(2):
                    cps[h01] = ps_c.tile([HD + 1, N], F32,
                                         tag=f"c{h01}", name=f"cps{h01}")
            for h01, at_t in ((0, ae), (1, ao)):
                h = 2 * pr + h01
                for nb in range(NB):
                    sl = slice(nb * 512, nb * 512 + 512)
                    nc.tensor.matmul(cps[h01][:, sl], v_sb[:, mt, h, :],
                                     at_t[:, sl],
                                     start=(mt == 0), stop=(mt == MT - 1))
            if mt == MT - 1:
                g = grp_of(2 * pr)
                if g != 5 and g not in dgs:
                    dgs[g] = stats.tile([6, N], F16, tag="dg", name=f"dg{g}")
                for h01 in range(2):
                    h = 2 * pr + h01
                    craw = p_craw.tile([HD + 1, N], F16,
                                       tag=f"cr{h % 6}", name="craw")
                    craws[h] = craw
                    nc.vector.tensor_copy(craw[:], cps[h01][:])
                    if g != 5:
                        nc.sync.dma_start(out=dgs[g][h - GROUPS[g][0]:
                                                     h - GROUPS[g][0] + 1, :],
                                          in_=craw[HD:HD + 1, :])
                if pr in (2, 4):
                    den_group(pr)

        pendings = []
        for pr in range(NPR):
            for mt in range(MT):
                msl = slice(mt * 128, mt * 128 + 128)
                pse = ps_s.tile([128, N], F32, tag="pse", name="pse")
                pso = ps_s.tile([128, N], F32, tag="pso", name="pso")
                for nb in range(NB):
                    sl = slice(nb * 512, nb * 512 + 512)
                    nc.tensor.matmul(pse[:, sl], k_sb[0:64, pr, msl],
                                     q_sb[0:64, pr, sl], start=True, stop=True)
                    nc.tensor.matmul(pso[:, sl], k_sb[64:128, pr, msl],
                                     q_sb[64:128, pr, sl], start=True, stop=True)
                ae = p_ae.tile([128, N], F16, tag="attnT", name="ae")
                ao = p_ae.tile([128, N], F16, tag="attnT", name="ao")
                nc.scalar.activation(out=ae[:], in_=pse[:], func=AF.Exp,
                                     scale=SOFT_SCALE)
                nc.scalar.activation(out=ao[:], in_=pso[:], func=AF.Exp,
                                     scale=SOFT_SCALE)
                pendings.append((pr, mt, ae, ao))
                if len(pendings) > 2:
                    emit_av(*pendings.pop(0))
        for p in pendings:
            emit_av(*p)
        ps_c.release()
        ps_s.release()
        p_recb.release()
        p_craw.release()
        p_ae.release()
        p_v.release()
        p_qk.release()

        # ------------- Phase 3+4: proj + LN1 + MLP + LN2 -------------
        p_r1 = tc.alloc_tile_pool(name="p_r1", bufs=1)
        r1_sb = p_r1.tile([128, DC, N], F16)
        p_y2 = tc.alloc_tile_pool(name="p_y2", bufs=1)
        p_h = tc.alloc_tile_pool(name="p_h", bufs=1)
        p_y16 = tc.alloc_tile_pool(name="p_y16", bufs=3)
        y2_sb = p_y2.tile([128, DC, N], F16)
        h_sb = p_h.tile([128, FC, N], F8)
        # ps_mm first so its four banks alias the score-psum banks (released
        # at the last exp read, ~2us before the attn@v accumulators drain) --
        # the proj up-front matmuls start that much earlier
        ps_mm = tc.alloc_tile_pool(name="ps_mm", bufs=4, space="PSUM")
        ps_ln = tc.alloc_tile_pool(name="ps_ln", bufs=1, space="PSUM")
        ps_ab = tc.alloc_tile_pool(name="ps_ab", bufs=1, space="PSUM")

        def ln_accum(st, src_sb, et, sl, first, last, sq_eng=None):
            """Fold chunk et of the pre-norm tensor into the LN sums. The s1/s2
            PSUM rows are allocated lazily on first call so the tag-rotating
            WAR dep lands after the previous user's finish chain is emitted.
            The square runs on GPSIMD (otherwise idle) for some chunks."""
            w = sl.stop - sl.start
            if "s1" not in st:
                st["s1"] = ps_ln.tile([1, 512], F32, tag="s1", name="s1")
                st["s2"] = ps_ln.tile([1, 512], F32, tag="s2", name="s2")
            sqt = p_u.tile([128, 512], F16, tag="sqt", name="sqt")
            eng = sq_eng or nc.vector
            eng.tensor_mul(sqt[:, 0:w], src_sb[:, et, sl], src_sb[:, et, sl])
            nc.tensor.matmul(st["s1"][:, 0:w], m16[:], src_sb[:, et, sl],
                             start=first, stop=last)
            nc.tensor.matmul(st["s2"][:, 0:w], p16[:], sqt[:, 0:w],
                             start=first, stop=last)

        def ln_finish(st, src_sb, sl, goff, boff, out_sb, out_f16=None,
                      out_dram=None, aff_eng=None, cast_eng=None,
                      split_out=False, out_f8=None):
            """Stats chain + affine for token slice sl. rsqrt(var+eps) is
            Exp(-0.5*Ln(.)) so everything stays in the nl-exp ACT set.
            Scalar chain is 3 ops (Square, Ln, Exp); the -1/D mean scale is
            folded into the negrow lhsT of the b-broadcast matmul. aff_eng
            picks the engine for the big affine ops (gpsimd for the
            non-critical LN2 half so the vector engine stays clear for the
            tail); cast_eng=scalar moves the a/b PSUM->SBUF cast off the
            vector engine; split_out pipelines the affine with the output
            DMA in two 3-chunk halves."""
            w = sl.stop - sl.start
            s1, s2 = st["s1"][:, 0:w], st["s2"][:, 0:w]
            ae_ = aff_eng or nc.vector
            t4 = stats.tile([1, 512], F32, tag="t4", name="t4")[:, 0:w]
            t2 = stats.tile([1, 512], F32, tag="t2", name="t2")[:, 0:w]
            t31f = stats.tile([1, 2, 512], F32R, tag="t31", name="t31")
            t3, t1 = t31f[:, 0, 0:w], t31f[:, 1, 0:w]  # a=1/std, b=-mu/std
            nc.scalar.activation(out=t4, in_=s1, func=AF.Square)  # mean^2
            nc.vector.tensor_sub(t2, s2, t4)                      # var
            nc.scalar.activation(out=t4, in_=t2, func=AF.Ln, bias=eps_t[:])
            nc.scalar.activation(out=t3, in_=t4, func=AF.Exp,
                                 scale=-0.5)                     # rsqrt(var+eps)
            nc.vector.tensor_mul(t1, s1, t3.bitcast(F32))
            abp = ps_ab.tile([128, 2, 512], F32, tag="abp", name="abp")
            if w <= 256:
                # one broadcast matmul for both a and b rows: out packs
                # [128, 2, w] inside a single PSUM bank
                abpv = abp[:, 0, 0:2 * w].rearrange("p (t w) -> p t w", t=2)
                nc.tensor.matmul(abpv, onesrow[:], t31f[:, :, 0:w],
                                 start=True, stop=True)
            else:
                abpv = abp[:, :, 0:w]
                nc.tensor.matmul(abp[:, 0, 0:w], onesrow[:], t3,
                                 start=True, stop=True)
                nc.tensor.matmul(abp[:, 1, 0:w], onesrow[:], t1,
                                 start=True, stop=True)
            abps = stats.tile([128, 2, 512], F16, tag=f"abps{sl.start}",
                              name="abps")
            if cast_eng is nc.scalar:
                nc.scalar.activation(out=abps[:, :, 0:w], in_=abpv,
                                     func=AF.Copy)
            else:
                (cast_eng or nc.vector).tensor_copy(abps[:, :, 0:w], abpv)

            def dma_half(half):
                cs = slice(3 * half, 3 * half + 3)
                nc.sync.dma_start(
                    out=out_dram[384 * half:384 * half + 384, sl].rearrange(
                        "(c p) n -> p c n", p=128),
                    in_=out_f16[:, cs, 0:w])

            if trivial_affine:
                # gamma==1, beta==0: out = src*a + b for all 6 chunks in
                # fused DVE ops using zero-stride broadcast of the per-token
                # scale/shift rows
                uall = p_sq.tile([128, DC, 512], F16, tag=f"uall{sl.start}",
                                 name="uall")
                if out_f8 is not None:
                    # produce the XS-scaled fp8 copy FIRST, in 2-chunk pieces
                    # (exactly the fc1 DR pair granularity) using a/b rows
                    # pre-scaled by XS on the scalar engine; the f16 copy is
                    # only needed ~25us later for the fc2 residual, so it is
                    # deferred to full-width ops afterwards.
                    abps8 = stats.tile([128, 2, 512], F16,
                                       tag=f"abps8{sl.start}", name="abps8")
                    nc.scalar.activation(out=abps8[:, :, 0:w], in_=abpv,
                                         func=AF.Copy, scale=XS)
                    for cs in (slice(0, 2), slice(2, 4), slice(4, 6)):
                        ab0x = abps8[:, 0:1, 0:w].to_broadcast([128, 2, w])
                        ab1x = abps8[:, 1:2, 0:w].to_broadcast([128, 2, w])
                        ae_.tensor_mul(uall[:, cs, 0:w], src_sb[:, cs, sl], ab0x)
                        ae_.tensor_add(out_f8[:, cs, sl], uall[:, cs, 0:w], ab1x)
                    ab0 = abps[:, 0:1, 0:w].to_broadcast([128, DC, w])
                    ab1 = abps[:, 1:2, 0:w].to_broadcast([128, DC, w])
                    ae_.tensor_mul(uall[:, :, 0:w], src_sb[:, :, sl], ab0)
                    ae_.tensor_add(out_sb[:, :, sl], uall[:, :, 0:w], ab1)
                else:
                    pieces = [slice(0, 3), slice(3, 6)] if split_out \
                        else [slice(0, DC)]
                    for pi, cs in enumerate(pieces):
                        nch = cs.stop - cs.start
                        ab0 = abps[:, 0:1, 0:w].to_broadcast([128, nch, w])
                        ab1 = abps[:, 1:2, 0:w].to_broadcast([128, nch, w])
                        ae_.tensor_mul(uall[:, cs, 0:w], src_sb[:, cs, sl], ab0)
                        if out_f16 is not None:
                            ae_.tensor_add(out_f16[:, cs, 0:w],
                                           uall[:, cs, 0:w], ab1)
                            if out_dram is not None and split_out:
                                dma_half(pi)
                        else:
                            ae_.tensor_add(out_sb[:, cs, sl],
                                           uall[:, cs, 0:w], ab1)
            else:
                for c in range(DC):
                    u = p_u.tile([128, 512], F16, tag="u", name="u")
                    ae_.tensor_mul(u[:, 0:w], src_sb[:, c, sl], abps[:, 0, 0:w])
                    ae_.tensor_add(u[:, 0:w], u[:, 0:w], abps[:, 1, 0:w])
                    dst = out_f16[:, c, 0:w] if out_f16 is not None else out_sb[:, c, sl]
                    ae_.tensor_scalar(out=dst, in0=u[:, 0:w],
                                      scalar1=consts_sb[:, goff + c:goff + c + 1],
                                      scalar2=consts_sb[:, boff + c:boff + c + 1],
                                      op0=OP.mult, op1=OP.add)
                    if out_f8 is not None:
                        ae_.tensor_scalar_mul(out_f8[:, c, sl],
                                              in0=out_sb[:, c, sl], scalar1=XS)
                    if out_dram is not None and split_out and c in (2, 5):
                        dma_half(c // 3)
            if out_dram is not None and not split_out:
                nc.sync.dma_start(
                    out=out_dram[:, sl].rearrange("(c p) n -> p c n", p=128),
                    in_=out_f16[:, :, 0:w])

        def proj_block(nb, st, pe_extras, defer_accums, fin_hook=None):
            """proj matmuls for token half nb. ctx chunks 0..4 are contracted
            for four et groups up front (the den-gated chunk 5 last) so the
            PE has queued work while the last denominator chain completes.
            pe_extras: deferred PE ops (prev half's tail accums) emitted
            after the up-front groups. Returns this half's deferred accum
            lambdas (all 6 when defer_accums, else the last 4)."""
            sl = slice(nb * 512, nb * 512 + 512)

            def drain(et, ps):
                t = p_u.tile([128, 512], F16, tag="pt", name="pt")
                nc.scalar.activation(out=t[:], in_=ps[:], func=AF.Identity,
                                     bias=consts_sb[:, CBPJ + et:CBPJ + et + 1],
                                     scale=1.0)
                nc.vector.tensor_add(r1_sb[:, et, sl], t[:], x16_sb[:, et, sl])

            pss = {}
            for et in range(4):
                pss[et] = ps_mm.tile([128, 512], F32, tag="mm", name="pspj")
            for c in range(5):
                for et in range(4):
                    nc.tensor.matmul(pss[et][:],
                                     wproj_sb[:, c, et * 128:(et + 1) * 128],
                                     ctx_sb[:, c, sl],
                                     start=(c == 0), stop=False)
            for fn in pe_extras:
                fn()
            for et in range(4):
                nc.tensor.matmul(pss[et][:],
                                 wproj_sb[:, 5, et * 128:(et + 1) * 128],
                                 ctx_sb[:, 5, sl], start=False, stop=True)
                drain(et, pss[et])
            for et in (4, 5):
                ps = ps_mm.tile([128, 512], F32, tag="mm", name="pspj")
                for c in range(DC):
                    nc.tensor.matmul(ps[:], wproj_sb[:, c, et * 128:(et + 1) * 128],
                                     ctx_sb[:, c, sl],
                                     start=(c == 0), stop=(c == DC - 1))
                drain(et, ps)
                if not defer_accums:
                    ln_accum(st, r1_sb, et - 4, sl, et - 4 == 0, False)
            if fin_hook is not None:
                fin_hook()
            first = 2 if not defer_accums else 0
            return [lambda e=e: ln_accum(st, r1_sb, e, sl, e == 0, e == DC - 1)
                    for e in range(first, DC)]

        def fc1_block(nb, extras):
            sl = slice(nb * 512, nb * 512 + 512)
            for ftg in range(FC):
                ps = ps_mm.tile([128, 512], F32, tag="mm", name="psf1")
                for j in range(DC // 2):
                    nc.tensor.matmul(ps[:],
                                     w1_sb[:, 2 * j:2 * j + 2,
                                           ftg * 128:(ftg + 1) * 128],
                                     x18_sb[:, 2 * j:2 * j + 2, sl],
                                     start=(j == 0), stop=(j == DC // 2 - 1),
                                     perf_mode=DR)
                nc.scalar.activation(out=h_sb[:, ftg, sl], in_=ps[:], func=AF.Gelu,
                                     bias=consts_sb[:, CBF1 + ftg:CBF1 + ftg + 1],
                                     scale=1.0 / (XS * WS))
                if ftg in extras:
                    extras[ftg]()

        def fc2_group(sl, et):
            w = sl.stop - sl.start
            ps = ps_mm.tile([128, 512], F32, tag="mm", name="psf2")
            for j in range(FC // 2):
                nc.tensor.matmul(ps[:, 0:w],
                                 w2_sb[:, 2 * j:2 * j + 2, et * 128:(et + 1) * 128],
                                 h_sb[:, 2 * j:2 * j + 2, sl],
                                 start=(j == 0), stop=(j == FC // 2 - 1),
                                 perf_mode=DR)
            t = p_u.tile([128, 512], F16, tag="ft", name="ft")
            nc.scalar.activation(out=t[:, 0:w], in_=ps[:, 0:w], func=AF.Identity,
                                 bias=consts_sb[:, CBF2 + et:CBF2 + et + 1],
                                 scale=1.0 / WS)
            nc.vector.tensor_add(y2_sb[:, et, sl], t[:, 0:w], x1_sb[:, et, sl])

        stA, stB, stC = {}, {}, {}
        sl0, sl1 = slice(0, 512), slice(512, 1024)
        accA = proj_block(0, stA, [den_group5_pe], defer_accums=False)
        # accums for et 2..5 of nb0 interleave into nb1's up-front groups and
        # fin(nb0) is emitted at nb1's block end; nb1 defers all its accums
        # into fc1(nb0) so the s1/s2 bank WAR lands after fin(nb0)'s reads,
        # and fin(nb1) lands mid-fc1(nb0) so x1(nb1) is ready well before
        # fc1(nb1) while its broadcast matmuls never stall the PE stream.
        accB = proj_block(1, stB, accA, defer_accums=True,
                          fin_hook=lambda: ln_finish(stA, r1_sb, sl0, CG1, CB1,
                                                     x1_sb, out_f8=x18_sb,
                                                     cast_eng=nc.scalar))

        def pair(fns):
            return lambda: [fn() for fn in fns]

        fc1_extras = {i: pair(accB[2 * i:2 * i + 2]) for i in range(3)}
        fc1_extras[15] = lambda: ln_finish(stB, r1_sb, sl1, CG1, CB1, x1_sb,
                                           out_f8=x18_sb, cast_eng=nc.scalar)
        fc1_block(0, fc1_extras)
        fc1_block(1, {})

        # fc2 nb0: full 512 block, internal defer-by-2 accums for et 0..3
        for et in range(DC):
            fc2_group(sl0, et)
            if et >= 2:
                ln_accum(stC, y2_sb, et - 2, sl0, et - 2 == 0, False)
        # fc2 nb1 runs as two 256-token quarters so quarter 0's LN2 finish
        # overlaps quarter 1's matmuls and only the last quarter's stats
        # chain is exposed at the very end.
        stD0, stD1 = {}, {}
        slq0, slq1 = slice(512, 768), slice(768, 1024)
        y16a = p_y16.tile([128, DC, 512], F16, tag="y16", name="y16a")
        for et in range(DC):
            fc2_group(slq0, et)
            if et == 0:
                ln_accum(stC, y2_sb, 4, sl0, False, False)
            elif et == 1:
                ln_accum(stC, y2_sb, 5, sl0, False, True)
            elif et == 2:
                ln_finish(stC, y2_sb, sl0, CG2, CB2, None, out_f16=y16a,
                          out_dram=yT16, cast_eng=nc.scalar)
            else:
                ln_accum(stD0, y2_sb, et - 3, slq0, et - 3 == 0, False,
                         sq_eng=nc.vector)
        y16b = p_y16.tile([128, DC, 512], F16, tag="y16", name="y16b")
        for et in range(DC):
            fc2_group(slq1, et)
            if et < 3:
                ln_accum(stD0, y2_sb, et + 3, slq0, False, et == 2,
                         sq_eng=nc.vector)
            elif et == 3:
                ln_finish(stD0, y2_sb, slq0, CG2, CB2, None, out_f16=y16b,
                          out_dram=yT16, cast_eng=nc.scalar)
            else:
                ln_accum(stD1, y2_sb, et - 4, slq1, et - 4 == 0, False,
                         sq_eng=nc.vector)
        for e in (2, 3, 4, 5):
            ln_accum(stD1, y2_sb, e, slq1, False, e == DC - 1, sq_eng=nc.vector)
        y16c = p_y16.tile([128, DC, 512], F16, tag="y16", name="y16c")
        ln_finish(stD1, y2_sb, slq1, CG2, CB2, None, out_f16=y16c,
                  out_dram=yT16, cast_eng=nc.scalar, split_out=True)
        ps_ab.release()
        ps_ln.release()
        ps_mm.release()
        dscr.release()
        p_y16.release()
        p_h.release()
        p_y2.release()
        p_r1.release()
        p_ctx.release()
        p_w1.release()
        p_w2.release()
        p_wproj.release()
        p_x16.release()
        p_u.release()
        p_sq.release()
        p_x1.release()
        stats.release()
        const.release()
    return nc


_NC_CACHE = {}


def _get_nc(trivial_affine=False):
    nc = _NC_CACHE.get(trivial_affine)
    if nc is None:
        nc = _build(trivial_affine)
        _split_excess_waits(nc)
        _NC_CACHE[trivial_affine] = nc
    return nc


def kernel(x, w_qkv, w_proj, b_proj, w_fc1, b_fc1, w_fc2, b_fc2,
           gamma1, beta1, gamma2, beta2):
    global LAST_RESULT
    x = np.asarray(x, dtype=np.float32)
    w_qkv = np.asarray(w_qkv, dtype=np.float32)
    w_proj = np.asarray(w_proj, dtype=np.float32)
    b_proj = np.asarray(b_proj, dtype=np.float32)
    w_fc1 = np.asarray(w_fc1, dtype=np.float32)
    b_fc1 = np.asarray(b_fc1, dtype=np.float32)
    w_fc2 = np.asarray(w_fc2, dtype=np.float32)
    b_fc2 = np.asarray(b_fc2, dtype=np.float32)
    gamma1 = np.asarray(gamma1, dtype=np.float32)
    beta1 = np.asarray(beta1, dtype=np.float32)
    gamma2 = np.asarray(gamma2, dtype=np.float32)
    beta2 = np.asarray(beta2, dtype=np.float32)

    F8NP = ml_dtypes.float8_e4m3
    wqkv_scaled = w_qkv.copy()
    wqkv_scaled[:D] *= HD ** -0.5                  # fold attention scale into Q
    wqkvT = np.ascontiguousarray((wqkv_scaled.T * WS).astype(F8NP))
    wprojT = np.ascontiguousarray(w_proj.T.astype(np.float16))
    wfc1T = np.ascontiguousarray((w_fc1.T * WS).astype(F8NP))
    wfc2T = np.ascontiguousarray((w_fc2.T * WS).astype(F8NP))

    def cols(v, nchunks):
        return np.ascontiguousarray(v.reshape(nchunks, 128).T)

    constsC = np.ascontiguousarray(np.hstack([
        cols(b_proj, DC), cols(b_fc1, FC), cols(b_fc2, DC),
        cols(gamma1, DC), cols(beta1, DC), cols(gamma2, DC), cols(beta2, DC),
    ]).astype(np.float32))

    shared = {
        "wqkvT": wqkvT, "wprojT": wprojT, "wfc1T": wfc1T, "wfc2T": wfc2T,
        "constsC": constsC,
    }
    in_maps = []
    for b in range(NCORES):
        m = dict(shared)
        xt = np.ascontiguousarray(x[b].T)
        m["xT16"] = xt.astype(np.float16)
        m["xT8"] = (xt * XS).astype(F8NP)
        in_maps.append(m)

    trivial = (np.all(gamma1 == 1.0) and np.all(beta1 == 0.0)
               and np.all(gamma2 == 1.0) and np.all(beta2 == 0.0))
    nc = _get_nc(trivial_affine=bool(trivial))
    LAST_RESULT = run_bass_kernel_spmd(nc, in_maps, list(range(NCORES)))
    out = np.stack([np.ascontiguousarray(LAST_RESULT.results[b]["yT16"].T)
                    for b in range(NCORES)])
    return out.astype(np.float32)
